# revision 1
# baseline (speedup 1.0000x reference)
"""GIN-style 5-layer GNN message passing on 8 Trainium2 NeuronCores.

Strategy (1D node-parallel):
  - Nodes partitioned contiguously across 8 cores (12500 each, padded to
    12544 = 98*128). Edges owned by their dst core.
  - Per layer: AllGather the per-core h shards into a full node table,
    dma_gather h[src] rows (512B each) in chunks of 6144, dma_scatter_add
    into two per-core accumulator buffers (two 3072-edge calls per chunk,
    dst indices unique within each call -- the DMA scatter-add loses
    updates when duplicate rows race within ~1k descriptors).
  - Edge embeddings never touch the edge stream: sum of incoming edge
    embeddings per node == counts[node, 0:9] @ etab[l], a tiny matmul.
    Self-loops likewise reduce to adding h[dst] once (done in the MLP).
  - GIN MLP (D->2D->relu->D) + BatchNorm folded into the second linear,
    computed per 128-node tile on the tensor engine.
"""
import sys
import numpy as np

sys.path.insert(0, "/opt/trn_rl_repo")

import concourse.bass as bass
import concourse.bacc as bacc
import concourse.tile as tile
import concourse.masks as masks
from concourse import mybir
from concourse.bass_utils import run_bass_kernel_spmd


class CFG:
    N = 100000          # total nodes
    D = 128             # feature dim
    L = 5               # layers
    NCORE = 8
    NOWN = 12500        # nodes per core
    NPAD = 12544        # padded nodes per core (98 * 128)
    NBLK = 4            # gather source windows
    CHUNK = 3072        # edges per chunk (= one scatter call)
    GCALL = 1024        # max idxs per dma_gather call (HW ring limit)
    CPB = 18            # chunks per (core, block)
    EPS = 1e-5

    @property
    def WIN(self):      # rows per gather window in h_full space
        return 2 * self.NPAD


    @property
    def NCH(self):
        return self.NBLK * self.CPB

    @property
    def ESLOT(self):
        return self.NCH * self.CHUNK

    @property
    def AGR(self):      # accumulator rows: NPAD + CHUNK trash + margin
        return self.NPAD + self.CHUNK + 128

    @property
    def NTILE(self):
        return self.NPAD // 128

    @property
    def HFULL(self):
        return self.NCORE * self.NPAD


def _fold_params(cfg, x_emb, etab, w1, b1, w2, b2, gamma, beta, bn_mean, bn_var):
    """Host-side parameter folding. Returns replicated device param arrays."""
    D, L = cfg.D, cfg.L
    f32 = np.float32
    x_emb = np.asarray(x_emb, np.float64)
    etab = np.asarray(etab, np.float64)
    w1 = np.asarray(w1, np.float64)
    b1 = np.asarray(b1, np.float64)
    w2 = np.asarray(w2, np.float64)
    b2 = np.asarray(b2, np.float64)
    gamma = np.asarray(gamma, np.float64)
    beta = np.asarray(beta, np.float64)
    bn_mean = np.asarray(bn_mean, np.float64)
    bn_var = np.asarray(bn_var, np.float64)

    xemb6 = np.zeros((8, D), f32)
    xemb6[0:3] = x_emb[0:3]
    xemb6[3:6] = x_emb[120:123]

    etab9 = np.zeros((L, 16, D), f32)
    etab9[:, 0:9, :] = etab

    w1t = np.ascontiguousarray(np.transpose(w1, (0, 2, 1))).astype(f32)  # [L,D,2D]
    b1t = np.ascontiguousarray(b1.reshape(L, 2, D).transpose(0, 2, 1)).astype(f32)  # [L,D,2]

    s = gamma / np.sqrt(bn_var + cfg.EPS)          # [L, D]
    t = beta - bn_mean * s
    w2f = w2 * s[:, :, None]                       # [L, D, 2D] rows scaled
    b2f = b2 * s + t                               # [L, D]
    # stationary chunks: w2s[l, p, k, m] = w2f[l, m, k*128 + p]
    w2s = np.ascontiguousarray(
        np.transpose(w2f.reshape(L, D, 2, D), (0, 3, 2, 1))
    ).astype(f32)                                   # [L, 128, 2, 128]
    b2t = b2f.astype(f32).reshape(L, D, 1)
    return dict(xemb6=xemb6, etab9=etab9, w1t=w1t, b1t=b1t, w2s=w2s, b2t=b2t)


def _wrap16(a):
    """Element i -> [i % 16, i // 16], replicated to 128 partitions."""
    assert len(a) % 16 == 0
    w = a.reshape(-1, 16).T
    return np.ascontiguousarray(np.tile(w, (8, 1)))


def _schedule_core(cfg, src_g, dst_l, rng):
    """Assign this core's edges (global src, local dst) to scatter slots.

    Returns gidx [ESLOT] (window-local gather idx) and sidx [ESLOT]
    (accumulator row). Slot = chunk (one scatter call): block-major,
    chunk-major; within a chunk edges are sorted by dst and padded with
    dummy edges (src 0 of the window, unique trash rows >= NPAD).
    Guarantees: dst unique within each chunk; all idx values < 2**15.
    """
    NOWN, NPAD, WIN = cfg.NOWN, cfg.NPAD, cfg.WIN
    S = cfg.CPB                          # slots (chunks) per block
    HALF = cfg.CHUNK                     # slot capacity
    q = src_g // NOWN
    src_l = q * NPAD + (src_g - q * NOWN)        # h_full row
    blk = src_l // WIN
    widx = src_l - blk * WIN                     # window-local row
    assert widx.max() < 2 ** 15

    gidx = np.zeros(cfg.ESLOT, np.int16)
    sidx = np.zeros(cfg.ESLOT, np.int16)
    # default: dummy pattern (src 0, trash rows NPAD + pos-in-chunk)
    pos_in_chunk = np.arange(cfg.ESLOT) % cfg.CHUNK
    sidx[:] = (NPAD + pos_in_chunk).astype(np.int16)

    for b in range(cfg.NBLK):
        m = blk == b
        eb_w = widx[m]
        eb_d = dst_l[m]
        n = len(eb_d)
        cap_total = S * HALF
        assert n <= cap_total - S, f"block {b} overflow: {n} > {cap_total - S}"
        # per-dst occurrence number
        order = np.argsort(eb_d, kind="stable")
        ds = eb_d[order]
        first = np.r_[True, ds[1:] != ds[:-1]]
        grp_start = np.flatnonzero(first)
        occ = np.arange(n) - np.repeat(grp_start, np.diff(np.r_[grp_start, n]))
        maxdeg = occ.max() + 1 if n else 0
        assert maxdeg <= S, f"node degree {maxdeg} exceeds {S} slots in block {b}"
        offs = rng.integers(0, S, size=NOWN)
        slot = (offs[ds] + occ) % S
        # capacity repair: move overflow edges to free slots without
        # breaking per-slot dst uniqueness.
        fill = np.bincount(slot, minlength=S)
        margin = HALF - 8  # slot capacity margin
        if fill.max() > margin:
            used = [set() for _ in range(S)]
            slot_l = slot.tolist()
            ds_l = ds.tolist()
            for i in range(n):
                used[slot_l[i]].add(ds_l[i])
            for s_over in range(S):
                while fill[s_over] > margin:
                    # pick an edge in s_over movable to the emptiest ok slot
                    moved = False
                    idxs = np.flatnonzero(slot == s_over)
                    for i in idxs:
                        d = ds_l[i]
                        cands = sorted(range(S), key=lambda x: fill[x])
                        for s_new in cands:
                            if fill[s_new] < margin and d not in used[s_new]:
                                used[s_over].discard(d)
                                used[s_new].add(d)
                                slot[i] = s_new
                                slot_l[i] = s_new
                                fill[s_over] -= 1
                                fill[s_new] += 1
                                moved = True
                                break
                        if moved:
                            break
                    assert moved, "capacity repair failed"
        # emit each slot
        for s_i in range(S):
            sel = slot == s_i
            w_s = eb_w[order][sel]
            d_s = ds[sel]
            k = len(d_s)
            assert k <= HALF
            assert len(np.unique(d_s)) == k, "duplicate dst within scatter call"
            so = np.argsort(d_s, kind="stable")
            base = (b * cfg.CPB + s_i) * cfg.CHUNK
            gidx[base:base + k] = w_s[so].astype(np.int16)
            sidx[base:base + k] = d_s[so].astype(np.int16)
            # rest of the half keeps the dummy pattern (already unique)
    return gidx, sidx


def _prepare_inputs(cfg, x, edge_index, edge_attr):
    """Per-core index/feature-side host prep."""
    x = np.asarray(x)
    src = np.asarray(edge_index[0], np.int64)
    dst = np.asarray(edge_index[1], np.int64)
    eb = np.asarray(edge_attr[:, 0], np.int64)
    ed = np.asarray(edge_attr[:, 1], np.int64)
    rng = np.random.default_rng(12345)

    per_core = []
    owner = dst // cfg.NOWN
    for r in range(cfg.NCORE):
        m = owner == r
        dst_l = dst[m] - r * cfg.NOWN
        gidx, sidx = _schedule_core(cfg, src[m], dst_l, rng)

        # counts[j, n]: incoming (incl self-loop) bond-type/direction counts
        countsT = np.zeros((16, cfg.NPAD), np.float32)
        np.add.at(countsT, (eb[m], dst_l), 1.0)
        np.add.at(countsT, (6 + ed[m], dst_l), 1.0)
        loc = np.arange(cfg.NOWN)
        countsT[4, loc] += 1.0   # self-loop bond type 4
        countsT[6, loc] += 1.0   # self-loop direction 0

        xohT = np.zeros((8, cfg.NPAD), np.float32)
        xl = np.asarray(x[r * cfg.NOWN:(r + 1) * cfg.NOWN], np.int64)
        xohT[xl[:, 0], loc] = 1.0
        xohT[3 + xl[:, 1], loc] += 1.0

        per_core.append(dict(
            gidx=_wrap16(gidx), sidx=_wrap16(sidx),
            countsT=countsT, xohT=xohT,
        ))
    return per_core


def _build_program(cfg):
    nc = bacc.Bacc(None, target_bir_lowering=False, debug=True)
    f32, i16 = mybir.dt.float32, mybir.dt.int16
    D, L = cfg.D, cfg.L

    # I/O
    gidx_in = nc.dram_tensor("gidx", [128, cfg.ESLOT // 16], i16, kind="ExternalInput")
    sidx_in = nc.dram_tensor("sidx", [128, cfg.ESLOT // 16], i16, kind="ExternalInput")
    countsT_in = nc.dram_tensor("countsT", [16, cfg.NPAD], f32, kind="ExternalInput")
    xohT_in = nc.dram_tensor("xohT", [8, cfg.NPAD], f32, kind="ExternalInput")
    xemb6_in = nc.dram_tensor("xemb6", [8, D], f32, kind="ExternalInput")
    etab9_in = nc.dram_tensor("etab9", [L, 16, D], f32, kind="ExternalInput")
    w1t_in = nc.dram_tensor("w1t", [L, D, 2 * D], f32, kind="ExternalInput")
    b1t_in = nc.dram_tensor("b1t", [L, D, 2], f32, kind="ExternalInput")
    w2s_in = nc.dram_tensor("w2s", [L, 128, 2, 128], f32, kind="ExternalInput")
    b2t_in = nc.dram_tensor("b2t", [L, D, 1], f32, kind="ExternalInput")
    out_ext = nc.dram_tensor("out", [cfg.NPAD, D], f32, kind="ExternalOutput")

    # internal DRAM
    hown = [nc.dram_tensor(f"hown{l}", [cfg.NPAD, D], f32) for l in range(L)]
    hfull = [nc.dram_tensor(f"hfull{l}", [cfg.HFULL, D], f32, addr_space="Shared")
             for l in range(L)]
    aggA = [nc.dram_tensor(f"aggA{l}", [cfg.AGR, D], f32) for l in range(L)]
    aggB = [nc.dram_tensor(f"aggB{l}", [cfg.AGR, D], f32) for l in range(L)]

    relu = mybir.ActivationFunctionType.Relu

    with tile.TileContext(nc) as tc:
        with (
            tc.tile_pool(name="const", bufs=1) as const_pool,
            tc.tile_pool(name="gather", bufs=3) as gather_pool,
            tc.tile_pool(name="mlp", bufs=3) as mlp_pool,
            tc.tile_pool(name="psum", bufs=1, space="PSUM") as psum_pool,
            tc.tile_pool(name="psum2", bufs=2, space="PSUM") as psum2_pool,
        ):
            # ---- resident constants ----
            ident = const_pool.tile([128, 128], f32, tag="ident")
            masks.make_identity(nc, ident[:, :])
            zero = const_pool.tile([128, 2048], f32, tag="zero")
            nc.gpsimd.memset(zero[:, :], 0.0)
            gidx_t = const_pool.tile([128, cfg.ESLOT // 16], i16, tag="gidx")
            sidx_t = const_pool.tile([128, cfg.ESLOT // 16], i16, tag="sidx")
            nc.sync.dma_start(gidx_t[:, :], gidx_in[:, :])
            nc.sync.dma_start(sidx_t[:, :], sidx_in[:, :])
            xemb6 = const_pool.tile([8, D], f32, tag="xemb6")
            nc.sync.dma_start(xemb6[:, :], xemb6_in[:, :])
            etab9 = [const_pool.tile([16, D], f32, tag=f"etab9_{l}", name=f"etab9_{l}") for l in range(L)]
            w1t = [const_pool.tile([D, 2 * D], f32, tag=f"w1t_{l}", name=f"w1t_{l}") for l in range(L)]
            b1t = [const_pool.tile([D, 2], f32, tag=f"b1t_{l}", name=f"b1t_{l}") for l in range(L)]
            w2s = [const_pool.tile([128, 2, 128], f32, tag=f"w2s_{l}", name=f"w2s_{l}") for l in range(L)]
            b2t = [const_pool.tile([D, 1], f32, tag=f"b2t_{l}", name=f"b2t_{l}") for l in range(L)]
            for l in range(L):
                nc.sync.dma_start(etab9[l][:, :], etab9_in[l])
                nc.sync.dma_start(w1t[l][:, :], w1t_in[l])
                nc.sync.dma_start(b1t[l][:, :], b1t_in[l])
                nc.sync.dma_start(w2s[l][:, :, :], w2s_in[l])
                nc.sync.dma_start(b2t[l][:, :], b2t_in[l])

            # ---- zero accumulators (scheduler overlaps with AG0/h0) ----
            for buf in [t for l in range(L) for t in (aggA[l], aggB[l])]:
                r0 = 0
                while r0 < cfg.AGR:
                    rows = min(2048, cfg.AGR - r0)
                    assert rows % 128 == 0
                    view = buf[r0:r0 + rows, :].rearrange(
                        "(i o) d -> i (o d)", i=128)
                    nc.sync.dma_start(view, zero[:, : rows * D // 128])
                    r0 += rows

            # ---- layer-0 node embedding: hown0 = onehot @ xemb6 ----
            for t in range(cfg.NTILE):
                cols = slice(t * 128, (t + 1) * 128)
                xoh_t = mlp_pool.tile([8, 128], f32, tag="xoh_t")
                nc.sync.dma_start(xoh_t[:, :], xohT_in[:, cols])
                h0p = psum_pool.tile([128, D], f32, tag="tp")
                nc.tensor.matmul(h0p[:, :], xoh_t[:, :], xemb6[:, :],
                                 start=True, stop=True)
                h0s = mlp_pool.tile([128, D], f32, tag="oS")
                nc.vector.tensor_copy(h0s[:, :], h0p[:, :])
                nc.sync.dma_start(hown[0][cols, :], h0s[:, :])

            # ---- layers ----
            for l in range(L):
                nc.gpsimd.collective_compute(
                    "AllGather", mybir.AluOpType.bypass,
                    ins=[hown[l][:, :]], outs=[hfull[l][:, :]],
                    replica_groups=[list(range(cfg.NCORE))],
                )
                # gather + scatter-add over edge chunks
                for c in range(cfg.NCH):
                    b = c // cfg.CPB
                    win = hfull[l][b * cfg.WIN:(b + 1) * cfg.WIN, :]
                    g = gather_pool.tile([128, cfg.CHUNK // 128, D], f32, tag="g")
                    ngc = cfg.CHUNK // cfg.GCALL
                    for gc in range(ngc):
                        gcols = cfg.GCALL // 128
                        ic = slice(c * (cfg.CHUNK // 16) + gc * (cfg.GCALL // 16),
                                   c * (cfg.CHUNK // 16) + (gc + 1) * (cfg.GCALL // 16))
                        nc.gpsimd.dma_gather(
                            g[:, gc * gcols:(gc + 1) * gcols, :], win,
                            gidx_t[:, ic], cfg.GCALL, cfg.GCALL, D, queue_num=0)
                    isl = slice(c * (cfg.CHUNK // 16), (c + 1) * (cfg.CHUNK // 16))
                    tgt = aggA[l] if c % 2 == 0 else aggB[l]
                    nc.gpsimd.dma_scatter_add(
                        tgt[:, :], g[:, :, :], sidx_t[:, isl],
                        cfg.CHUNK, cfg.CHUNK, D, queue_num=0)

                # MLP per 128-node tile
                for t in range(cfg.NTILE):
                    rows = slice(t * 128, (t + 1) * 128)
                    cols = slice(t * 128, (t + 1) * 128)
                    a = mlp_pool.tile([128, D], f32, tag="a")
                    bb = mlp_pool.tile([128, D], f32, tag="bb")
                    hw = mlp_pool.tile([128, D], f32, tag="hw")
                    nc.sync.dma_start(a[:, :], aggA[l][rows, :])
                    nc.sync.dma_start(bb[:, :], aggB[l][rows, :])
                    nc.sync.dma_start(hw[:, :], hown[l][rows, :])
                    nc.vector.tensor_add(a[:, :], a[:, :], bb[:, :])
                    nc.vector.tensor_add(a[:, :], a[:, :], hw[:, :])
                    # feature-major: tp = a.T (+ einit via counts matmul)
                    tp = psum_pool.tile([128, D], f32, tag="tp")
                    nc.tensor.transpose(tp[:, :], a[:, :], ident[:, :])
                    cnt_t = mlp_pool.tile([16, 128], f32, tag="cnt_t")
                    nc.sync.dma_start(cnt_t[:, :], countsT_in[:, cols])
                    ei = psum_pool.tile([128, D], f32, tag="ei")
                    nc.tensor.matmul(ei[:, :], etab9[l][:, :], cnt_t[:, :],
                                     start=True, stop=True)
                    eiS = mlp_pool.tile([128, D], f32, tag="eiS")
                    nc.vector.tensor_copy(eiS[:, :], ei[:, :])
                    tS = mlp_pool.tile([128, D], f32, tag="tS")
                    nc.vector.tensor_add(tS[:, :], tp[:, :], eiS[:, :])
                    # mm1 + relu + b1
                    hm = psum2_pool.tile([128, 2, 128], f32, tag="hm")
                    hmS = mlp_pool.tile([128, 2, 128], f32, tag="hmS")
                    for j in range(2):
                        nc.tensor.matmul(hm[:, j, :], w1t[l][:, j * 128:(j + 1) * 128],
                                         tS[:, :], start=True, stop=True)
                        nc.scalar.activation(hmS[:, j, :], hm[:, j, :], relu,
                                             bias=b1t[l][:, j:j + 1])
                    # mm2 accumulate + bias (+ relu if not last layer)
                    h2p = psum_pool.tile([128, D], f32, tag="h2p")
                    for j in range(2):
                        nc.tensor.matmul(h2p[:, :], w2s[l][:, j, :], hmS[:, j, :],
                                         start=(j == 0), stop=(j == 1))
                    h2S = mlp_pool.tile([128, D], f32, tag="h2S")
                    if l < L - 1:
                        nc.scalar.activation(h2S[:, :], h2p[:, :], relu,
                                             bias=b2t[l][:, 0:1])
                    else:
                        nc.vector.tensor_scalar_add(h2S[:, :], h2p[:, :],
                                                    b2t[l][:, 0:1])
                    # back to node-major and store
                    op = psum_pool.tile([128, D], f32, tag="op")
                    nc.tensor.transpose(op[:, :], h2S[:, :], ident[:, :])
                    oS = mlp_pool.tile([128, D], f32, tag="oS")
                    nc.vector.tensor_copy(oS[:, :], op[:, :])
                    dst_t = out_ext if l == L - 1 else hown[l + 1]
                    nc.sync.dma_start(dst_t[rows, :], oS[:, :])

    nc.finalize()
    return nc


_CACHE = {}


def _get_program(cfg):
    key = (cfg.N, cfg.CHUNK, cfg.CPB)
    if key not in _CACHE:
        _CACHE[key] = _build_program(cfg)
    return _CACHE[key]


def build_in_maps(cfg, inputs):
    params = _fold_params(
        cfg, inputs["x_emb"], inputs["etab"], inputs["w1"], inputs["b1"],
        inputs["w2"], inputs["b2"], inputs["gamma"], inputs["beta"],
        inputs["bn_mean"], inputs["bn_var"])
    per_core = _prepare_inputs(cfg, inputs["x"], inputs["edge_index"],
                               inputs["edge_attr"])
    in_maps = []
    for r in range(cfg.NCORE):
        m = dict(per_core[r])
        m.update({k: np.ascontiguousarray(v) for k, v in params.items()})
        in_maps.append(m)
    return in_maps


def kernel(**inputs) -> np.ndarray:
    cfg = CFG()
    nc = _get_program(cfg)
    in_maps = build_in_maps(cfg, inputs)
    res = run_bass_kernel_spmd(nc, in_maps, list(range(cfg.NCORE)))
    out = np.empty((cfg.N, cfg.D), np.float32)
    for r in range(cfg.NCORE):
        out[r * cfg.NOWN:(r + 1) * cfg.NOWN] = res.results[r]["out"][:cfg.NOWN]
    return out



# revision 6
# speedup vs baseline: 1.9108x; 1.9108x over previous
"""GIN-style 5-layer GNN message passing on 8 Trainium2 NeuronCores.

Strategy v2 (1D node-parallel, scatter-free):
  - Nodes partitioned contiguously across 8 cores (12500 each, padded to
    12544 = 98*128). Edges owned by their dst core.
  - Per layer: AllGather the per-core h shards (bf16) into a full node
    table in DRAM; dma_gather h[src] rows per edge (256B each) in
    dst-tile-sorted order, 2560 idxs per call.
  - Aggregation runs on the tensor engine, not the DMA scatter path:
    per 128-edge group (all edges of one dst tile), a one-hot fp8
    selection matrix contracts the gathered bf16 messages into a PSUM
    accumulator (out[dst, feat] += sel^T @ msg).  The self-loop term is
    the bank-filling first matmul (identity x h_own slab, start=True);
    edge embeddings fold into a counts matmul (counts^T @ etab).
  - GIN MLP (D->2D->relu->D) + BatchNorm folded into the second linear,
    bf16 weights, computed per 128-node tile on the tensor engine.
"""
import sys
import numpy as np

sys.path.insert(0, "/opt/trn_rl_repo")

import ml_dtypes
import concourse.bass as bass
import concourse.bacc as bacc
import concourse.tile as tile
import concourse.masks as masks
from concourse import mybir
from concourse.bass_utils import run_bass_kernel_spmd


class CFG:
    N = 100000          # total nodes
    D = 128             # feature dim
    L = 5               # layers
    NCORE = 8
    NOWN = 12500        # nodes per core
    NPAD = 12544        # padded nodes per core (98 * 128)
    NBLK = 4            # gather source windows (int16 idx limit)
    TPB = 4             # dst tiles per block
    CAP = 640           # slot capacity per (tile, window) cell; 5 groups
    EPS = 1e-5

    @property
    def WIN(self):      # rows per gather window in h_full space
        return 2 * self.NPAD

    @property
    def NTILE(self):
        return self.NPAD // 128

    @property
    def HFULL(self):
        return self.NCORE * self.NPAD

    @property
    def GPC(self):      # groups per cell
        return self.CAP // 128

    @property
    def blocks(self):   # list of tile-index lists
        t = list(range(self.NTILE))
        return [t[i:i + self.TPB] for i in range(0, self.NTILE, self.TPB)]

    @property
    def TOTSLOT(self):
        return self.NTILE * self.NBLK * self.CAP

    @property
    def TOTGRP(self):
        return self.TOTSLOT // 128


def _f8(a):
    return np.asarray(a, np.float32).astype(ml_dtypes.float8_e4m3fn)


def _bf(a):
    return np.asarray(a, np.float32).astype(ml_dtypes.bfloat16)


def _fold_params(cfg, x_emb, etab, w1, b1, w2, b2, gamma, beta, bn_mean, bn_var):
    """Host-side parameter folding. Returns replicated device param arrays."""
    D, L = cfg.D, cfg.L
    x_emb = np.asarray(x_emb, np.float64)
    etab = np.asarray(etab, np.float64)
    w1 = np.asarray(w1, np.float64)
    b1 = np.asarray(b1, np.float64)
    w2 = np.asarray(w2, np.float64)
    b2 = np.asarray(b2, np.float64)
    gamma = np.asarray(gamma, np.float64)
    beta = np.asarray(beta, np.float64)
    bn_mean = np.asarray(bn_mean, np.float64)
    bn_var = np.asarray(bn_var, np.float64)

    xemb6 = np.zeros((8, D), np.float64)
    xemb6[0:3] = x_emb[0:3]
    xemb6[3:6] = x_emb[120:123]

    etab9 = np.zeros((L, 16, D), np.float64)
    etab9[:, 0:9, :] = etab

    w1t = np.ascontiguousarray(np.transpose(w1, (0, 2, 1)))          # [L,D,2D]
    b1t = np.ascontiguousarray(
        b1.reshape(L, 2, D).transpose(0, 2, 1)).astype(np.float32)   # [L,D,2]

    s = gamma / np.sqrt(bn_var + cfg.EPS)          # [L, D]
    t = beta - bn_mean * s
    w2f = w2 * s[:, :, None]                       # [L, D, 2D] rows scaled
    b2f = b2 * s + t                               # [L, D]
    # stationary chunks: w2s[l, p, k, m] = w2f[l, m, k*128 + p]
    w2s = np.ascontiguousarray(
        np.transpose(w2f.reshape(L, D, 2, D), (0, 3, 2, 1)))         # [L,128,2,128]
    b2t = b2f.astype(np.float32).reshape(L, D, 1)
    return dict(xemb6=_bf(xemb6), etab9=_bf(etab9), w1t=_bf(w1t), b1t=b1t,
                w2s=_bf(w2s), b2t=b2t)


def _wrap16(a):
    """Element i -> [i % 16, i // 16], replicated to 128 partitions."""
    assert len(a) % 16 == 0
    w = a.reshape(-1, 16).T
    return np.ascontiguousarray(np.tile(w, (8, 1)))


def _cell_bases(cfg):
    """slot base for each (tile, window) cell, matching the call layout:
    for each block B, for each window w, tiles of B consecutively."""
    bases = np.zeros((cfg.NTILE, cfg.NBLK), np.int64)
    off = 0
    for blk in cfg.blocks:
        for w in range(cfg.NBLK):
            for t in blk:
                bases[t, w] = off
                off += cfg.CAP
    assert off == cfg.TOTSLOT
    return bases


def _schedule_core(cfg, src_g, dst_l):
    """Assign this core's edges (global src, local dst) to gather slots.

    Returns gidx [TOTSLOT] int16 (window-local gather idx; 0 for padding)
    and selT [128, TOTGRP, 128] fp8 one-hot matrices (zero rows for pads).
    """
    q = src_g // cfg.NOWN
    src_row = q * cfg.NPAD + (src_g - q * cfg.NOWN)
    w = src_row // cfg.WIN
    widx = (src_row - w * cfg.WIN).astype(np.int64)
    assert widx.max() < 2 ** 15
    t = dst_l // 128
    j = dst_l % 128

    cell = t * cfg.NBLK + w
    order = np.argsort(cell, kind="stable")
    cell_s = cell[order]
    counts = np.bincount(cell_s, minlength=cfg.NTILE * cfg.NBLK)
    assert counts.max() <= cfg.CAP, f"cell overflow: {counts.max()} > {cfg.CAP}"
    bases = _cell_bases(cfg).reshape(-1)
    cell_starts = np.zeros(cfg.NTILE * cfg.NBLK + 1, np.int64)
    np.cumsum(counts, out=cell_starts[1:])
    pos_in_cell = np.arange(len(cell_s)) - cell_starts[cell_s]
    slot = bases[cell_s] + pos_in_cell

    gidx = np.zeros(cfg.TOTSLOT, np.int16)
    gidx[slot] = widx[order].astype(np.int16)
    selT = np.zeros((128, cfg.TOTGRP, 128), ml_dtypes.float8_e4m3fn)
    selT[slot % 128, slot // 128, j[order]] = 1.0
    return gidx, selT


def _prepare_inputs(cfg, x, edge_index, edge_attr):
    """Per-core index/feature-side host prep."""
    x = np.asarray(x)
    src = np.asarray(edge_index[0], np.int64)
    dst = np.asarray(edge_index[1], np.int64)
    eb = np.asarray(edge_attr[:, 0], np.int64)
    ed = np.asarray(edge_attr[:, 1], np.int64)

    per_core = []
    owner = dst // cfg.NOWN
    for r in range(cfg.NCORE):
        m = owner == r
        dst_l = dst[m] - r * cfg.NOWN
        gidx, selT = _schedule_core(cfg, src[m], dst_l)

        # counts[j, n]: incoming (incl self-loop) bond-type/direction counts
        countsT = np.zeros((16, cfg.NPAD), np.float32)
        np.add.at(countsT, (eb[m], dst_l), 1.0)
        np.add.at(countsT, (6 + ed[m], dst_l), 1.0)
        loc = np.arange(cfg.NOWN)
        countsT[4, loc] += 1.0   # self-loop bond type 4
        countsT[6, loc] += 1.0   # self-loop direction 0

        xohT = np.zeros((8, cfg.NPAD), np.float32)
        xl = np.asarray(x[r * cfg.NOWN:(r + 1) * cfg.NOWN], np.int64)
        xohT[xl[:, 0], loc] = 1.0
        xohT[3 + xl[:, 1], loc] += 1.0

        per_core.append(dict(
            gidx=_wrap16(gidx), selT=selT,
            countsT=_bf(countsT), xohT=_bf(xohT),
        ))
    return per_core


def _build_program(cfg):
    nc = bacc.Bacc(None, target_bir_lowering=False, debug=True)
    f32, bf16, i16 = mybir.dt.float32, mybir.dt.bfloat16, mybir.dt.int16
    fp8 = mybir.dt.float8e4
    D, L = cfg.D, cfg.L
    GPC = cfg.GPC

    # I/O
    gidx_in = nc.dram_tensor("gidx", [128, cfg.TOTSLOT // 16], i16,
                             kind="ExternalInput")
    selT_in = nc.dram_tensor("selT", [128, cfg.TOTGRP, 128], fp8,
                             kind="ExternalInput")
    countsT_in = nc.dram_tensor("countsT", [16, cfg.NPAD], bf16,
                                kind="ExternalInput")
    xohT_in = nc.dram_tensor("xohT", [8, cfg.NPAD], bf16, kind="ExternalInput")
    xemb6_in = nc.dram_tensor("xemb6", [8, D], bf16, kind="ExternalInput")
    etab9_in = nc.dram_tensor("etab9", [L, 16, D], bf16, kind="ExternalInput")
    w1t_in = nc.dram_tensor("w1t", [L, D, 2 * D], bf16, kind="ExternalInput")
    b1t_in = nc.dram_tensor("b1t", [L, D, 2], f32, kind="ExternalInput")
    w2s_in = nc.dram_tensor("w2s", [L, 128, 2, 128], bf16, kind="ExternalInput")
    b2t_in = nc.dram_tensor("b2t", [L, D, 1], f32, kind="ExternalInput")
    out_ext = nc.dram_tensor("out", [cfg.NPAD, D], f32, kind="ExternalOutput")

    # internal DRAM (shared across layers; layers are serial)
    hown_d = nc.dram_tensor("hown", [cfg.NPAD, D], bf16)
    hfull = nc.dram_tensor("hfull", [cfg.HFULL, D], bf16, addr_space="Shared")

    relu = mybir.ActivationFunctionType.Relu

    with tile.TileContext(nc) as tc:
        with (
            tc.tile_pool(name="const", bufs=1) as const_pool,
            tc.tile_pool(name="gather", bufs=4) as gather_pool,
            tc.tile_pool(name="sel", bufs=4) as sel_pool,
            tc.tile_pool(name="mlp", bufs=3) as mlp_pool,
            tc.tile_pool(name="aggp", bufs=2, space="PSUM") as agg_pool,
            tc.tile_pool(name="psA", bufs=1, space="PSUM") as psA_pool,
            tc.tile_pool(name="psB", bufs=2, space="PSUM") as psB_pool,
        ):
            # ---- resident constants ----
            identf = const_pool.tile([128, 128], f32, tag="identf")
            masks.make_identity(nc, identf[:, :])
            identb = const_pool.tile([128, 128], bf16, tag="identb")
            nc.vector.tensor_copy(identb[:, :], identf[:, :])
            gidx_t = const_pool.tile([128, cfg.TOTSLOT // 16], i16, tag="gidx")
            nc.sync.dma_start(gidx_t[:, :], gidx_in[:, :])
            cntT = const_pool.tile([16, cfg.NPAD], bf16, tag="cntT")
            nc.sync.dma_start(cntT[:, :], countsT_in[:, :])
            hown_sb = const_pool.tile([128, cfg.NTILE, 128], bf16, tag="hown_sb")
            xemb6 = const_pool.tile([8, D], bf16, tag="xemb6")
            nc.sync.dma_start(xemb6[:, :], xemb6_in[:, :])
            etab9 = [const_pool.tile([16, D], bf16, tag=f"etab9_{l}",
                                     name=f"etab9_{l}") for l in range(L)]
            w1t = [const_pool.tile([D, 2 * D], bf16, tag=f"w1t_{l}",
                                   name=f"w1t_{l}") for l in range(L)]
            b1t = [const_pool.tile([D, 2], f32, tag=f"b1t_{l}",
                                   name=f"b1t_{l}") for l in range(L)]
            w2s = [const_pool.tile([128, 2, 128], bf16, tag=f"w2s_{l}",
                                   name=f"w2s_{l}") for l in range(L)]
            b2t = [const_pool.tile([D, 1], f32, tag=f"b2t_{l}",
                                   name=f"b2t_{l}") for l in range(L)]
            for l in range(L):
                nc.sync.dma_start(etab9[l][:, :], etab9_in[l])
                nc.sync.dma_start(w1t[l][:, :], w1t_in[l])
                nc.sync.dma_start(b1t[l][:, :], b1t_in[l])
                nc.sync.dma_start(w2s[l][:, :, :], w2s_in[l])
                nc.sync.dma_start(b2t[l][:, :], b2t_in[l])

            # ---- layer-0 node embedding: h0 = onehot @ xemb6 ----
            for t in range(cfg.NTILE):
                cols = slice(t * 128, (t + 1) * 128)
                xoh_t = mlp_pool.tile([8, 128], bf16, tag="xoh_t")
                nc.sync.dma_start(xoh_t[:, :], xohT_in[:, cols])
                h0p = psA_pool.tile([128, D], f32, tag="tp")
                nc.tensor.matmul(h0p[:, :], xoh_t[:, :], xemb6[:, :],
                                 start=True, stop=True)
                nc.vector.tensor_copy(hown_sb[:, t, :], h0p[:, :])
                nc.sync.dma_start(hown_d[cols, :], hown_sb[:, t, :])

            # ---- layers ----
            for l in range(L):
                nc.gpsimd.collective_compute(
                    "AllGather", mybir.AluOpType.bypass,
                    ins=[hown_d[:, :]], outs=[hfull[:, :]],
                    replica_groups=[list(range(cfg.NCORE))],
                )
                base = 0
                for bi, blk in enumerate(cfg.blocks):
                    nt = len(blk)
                    ng_call = nt * GPC          # groups per (block, window)
                    nidx = nt * cfg.CAP
                    # agg bank: filled+zeroed by the self-loop identity mm
                    agg = agg_pool.tile([128, cfg.TPB, 128], f32, tag="agg")
                    nc.tensor.matmul(
                        agg[:, 0:nt, :], identb[:, :],
                        hown_sb[:, blk[0]:blk[0] + nt, :],
                        start=True, stop=False, skip_group_check=True)
                    # edge-embedding init: counts^T @ etab
                    for i, t in enumerate(blk):
                        nc.tensor.matmul(
                            agg[:, i, :], cntT[:, t * 128:(t + 1) * 128],
                            etab9[l][:, :],
                            start=False, stop=False, skip_group_check=True)
                    for w in range(cfg.NBLK):
                        gbuf = gather_pool.tile([128, cfg.TPB * GPC, D], bf16,
                                                tag="g")
                        ic = slice(base // 16, (base + nidx) // 16)
                        nc.gpsimd.dma_gather(
                            gbuf[:, 0:ng_call, :],
                            hfull[w * cfg.WIN:(w + 1) * cfg.WIN, :],
                            gidx_t[:, ic], nidx, nidx, D,
                            single_packet=False, queue_num=0)
                        sel_t = sel_pool.tile([128, cfg.TPB * GPC, 128], fp8,
                                              tag="sel")
                        gsl = slice(base // 128, (base + nidx) // 128)
                        nc.sync.dma_start(sel_t[:, 0:ng_call, :],
                                          selT_in[:, gsl, :])
                        for g in range(ng_call):
                            last = (w == cfg.NBLK - 1) and (g == ng_call - 1)
                            nc.tensor.matmul(
                                agg[:, g // GPC, :], sel_t[:, g, :],
                                gbuf[:, g, :],
                                start=False, stop=last, skip_group_check=True)
                        base += nidx

                    # ---- MLP per tile of this block ----
                    # one whole-bank copy: depends on every slice's last
                    # matmul, so the PE is done with this bank before DVE
                    # reads it (PE-W + DVE-R same bank is a HW hazard)
                    aS = mlp_pool.tile([128, cfg.TPB, D], bf16, tag="aS")
                    nc.vector.tensor_copy(aS[:, 0:nt, :], agg[:, 0:nt, :])
                    for i, t in enumerate(blk):
                        tp = psA_pool.tile([128, D], bf16, tag="tpb")
                        nc.tensor.transpose(tp[:, :], aS[:, i, :], identb[:, :])
                        tS = mlp_pool.tile([128, D], bf16, tag="tS")
                        nc.vector.tensor_copy(tS[:, :], tp[:, :])
                        # mm1 + relu + b1
                        hm = psB_pool.tile([128, 2, 128], f32, tag="hm")
                        hmS = mlp_pool.tile([128, 2, 128], bf16, tag="hmS")
                        for j in range(2):
                            nc.tensor.matmul(hm[:, j, :],
                                             w1t[l][:, j * 128:(j + 1) * 128],
                                             tS[:, :], start=True, stop=True)
                            nc.scalar.activation(hmS[:, j, :], hm[:, j, :],
                                                 relu, bias=b1t[l][:, j:j + 1])
                        # mm2 accumulate + bias (+ relu if not last layer)
                        h2p = psA_pool.tile([128, D], f32, tag="h2p")
                        for j in range(2):
                            nc.tensor.matmul(h2p[:, :], w2s[l][:, j, :],
                                             hmS[:, j, :],
                                             start=(j == 0), stop=(j == 1))
                        if l < L - 1:
                            h2S = mlp_pool.tile([128, D], bf16, tag="h2S")
                            nc.scalar.activation(h2S[:, :], h2p[:, :], relu,
                                                 bias=b2t[l][:, 0:1])
                            op = psA_pool.tile([128, D], bf16, tag="opb")
                            nc.tensor.transpose(op[:, :], h2S[:, :],
                                                identb[:, :])
                            nc.vector.tensor_copy(hown_sb[:, t, :], op[:, :])
                            nc.sync.dma_start(
                                hown_d[t * 128:(t + 1) * 128, :],
                                hown_sb[:, t, :])
                        else:
                            h2S = mlp_pool.tile([128, D], f32, tag="h2Sf")
                            nc.vector.tensor_scalar_add(h2S[:, :], h2p[:, :],
                                                        b2t[l][:, 0:1])
                            op = psA_pool.tile([128, D], f32, tag="tp")
                            nc.tensor.transpose(op[:, :], h2S[:, :],
                                                identf[:, :])
                            oS = mlp_pool.tile([128, D], f32, tag="oSf")
                            nc.vector.tensor_copy(oS[:, :], op[:, :])
                            nc.sync.dma_start(
                                out_ext[t * 128:(t + 1) * 128, :], oS[:, :])
                assert base == cfg.TOTSLOT

    nc.finalize()
    return nc


_CACHE = {}


def _get_program(cfg):
    key = (cfg.N, cfg.CAP, cfg.TPB)
    if key not in _CACHE:
        _CACHE[key] = _build_program(cfg)
    return _CACHE[key]


def build_in_maps(cfg, inputs):
    params = _fold_params(
        cfg, inputs["x_emb"], inputs["etab"], inputs["w1"], inputs["b1"],
        inputs["w2"], inputs["b2"], inputs["gamma"], inputs["beta"],
        inputs["bn_mean"], inputs["bn_var"])
    per_core = _prepare_inputs(cfg, inputs["x"], inputs["edge_index"],
                               inputs["edge_attr"])
    in_maps = []
    for r in range(cfg.NCORE):
        m = dict(per_core[r])
        m.update({k: np.ascontiguousarray(v) for k, v in params.items()})
        in_maps.append(m)
    return in_maps


def kernel(**inputs) -> np.ndarray:
    cfg = CFG()
    nc = _get_program(cfg)
    in_maps = build_in_maps(cfg, inputs)
    res = run_bass_kernel_spmd(nc, in_maps, list(range(cfg.NCORE)))
    out = np.empty((cfg.N, cfg.D), np.float32)
    for r in range(cfg.NCORE):
        out[r * cfg.NOWN:(r + 1) * cfg.NOWN] = res.results[r]["out"][:cfg.NOWN]
    return out


# revision 20
# speedup vs baseline: 1.9483x; 1.0196x over previous
"""GIN-style 5-layer GNN message passing on 8 Trainium2 NeuronCores.

Strategy v2 (1D node-parallel, scatter-free):
  - Nodes partitioned contiguously across 8 cores (12500 each, padded to
    12544 = 98*128). Edges owned by their dst core.
  - Per layer: AllGather the per-core h shards (bf16) into a full node
    table in DRAM; dma_gather h[src] rows per edge (256B each) in
    dst-tile-sorted order, 2560 idxs per call.
  - Aggregation runs on the tensor engine, not the DMA scatter path:
    per 128-edge group (all edges of one dst tile), a one-hot fp8
    selection matrix contracts the gathered bf16 messages into a PSUM
    accumulator (out[dst, feat] += sel^T @ msg).  The self-loop term is
    the bank-filling first matmul (identity x h_own slab, start=True);
    edge embeddings fold into a counts matmul (counts^T @ etab).
  - GIN MLP (D->2D->relu->D) + BatchNorm folded into the second linear,
    bf16 weights, computed per 128-node tile on the tensor engine.
"""
import sys
import numpy as np

sys.path.insert(0, "/opt/trn_rl_repo")

import ml_dtypes
import concourse.bass as bass
import concourse.bacc as bacc
import concourse.tile as tile
import concourse.masks as masks
from concourse import mybir
from concourse.bass_utils import run_bass_kernel_spmd


class CFG:
    DEBUG_TAPS = False
    TAPS = ()
    SPLIT_AG = True
    N = 100000          # total nodes
    D = 128             # feature dim
    L = 5               # layers
    NCORE = 8
    NOWN = 12500        # nodes per core
    NPAD = 12544        # padded nodes per core (98 * 128)
    NBLK = 4            # gather source windows (int16 idx limit)
    TPB = 4             # dst tiles per block
    CAP = 640           # slot capacity per (tile, window) cell; 5 groups
    EPS = 1e-5

    @property
    def WIN(self):      # rows per gather window in h_full space
        return 2 * self.NPAD

    @property
    def NTILE(self):
        return self.NPAD // 128

    @property
    def HFULL(self):
        return self.NCORE * self.NPAD

    @property
    def GPC(self):      # groups per cell
        return self.CAP // 128

    @property
    def blocks(self):   # list of tile-index lists
        t = list(range(self.NTILE))
        return [t[i:i + self.TPB] for i in range(0, self.NTILE, self.TPB)]

    @property
    def TOTSLOT(self):
        return self.NTILE * self.NBLK * self.CAP

    @property
    def TOTGRP(self):
        return self.TOTSLOT // 128


def _f8(a):
    return np.asarray(a, np.float32).astype(ml_dtypes.float8_e4m3fn)


def _bf(a):
    return np.asarray(a, np.float32).astype(ml_dtypes.bfloat16)


def _fold_params(cfg, x_emb, etab, w1, b1, w2, b2, gamma, beta, bn_mean, bn_var):
    """Host-side parameter folding. Returns replicated device param arrays."""
    D, L = cfg.D, cfg.L
    x_emb = np.asarray(x_emb, np.float64)
    etab = np.asarray(etab, np.float64)
    w1 = np.asarray(w1, np.float64)
    b1 = np.asarray(b1, np.float64)
    w2 = np.asarray(w2, np.float64)
    b2 = np.asarray(b2, np.float64)
    gamma = np.asarray(gamma, np.float64)
    beta = np.asarray(beta, np.float64)
    bn_mean = np.asarray(bn_mean, np.float64)
    bn_var = np.asarray(bn_var, np.float64)

    xemb6 = np.zeros((8, D), np.float64)
    xemb6[0:3] = x_emb[0:3]
    xemb6[3:6] = x_emb[120:123]

    etab9 = np.zeros((L, 16, D), np.float64)
    etab9[:, 0:9, :] = etab

    w1t = np.ascontiguousarray(np.transpose(w1, (0, 2, 1)))          # [L,D,2D]
    b1t = np.ascontiguousarray(
        b1.reshape(L, 2, D).transpose(0, 2, 1)).astype(np.float32)   # [L,D,2]

    s = gamma / np.sqrt(bn_var + cfg.EPS)          # [L, D]
    t = beta - bn_mean * s
    w2f = w2 * s[:, :, None]                       # [L, D, 2D] rows scaled
    b2f = b2 * s + t                               # [L, D]
    # stationary chunks: w2s[l, p, k, m] = w2f[l, m, k*128 + p]
    w2s = np.ascontiguousarray(
        np.transpose(w2f.reshape(L, D, 2, D), (0, 3, 2, 1)))         # [L,128,2,128]
    b2t = b2f.astype(np.float32).reshape(L, D, 1)
    return dict(xemb6=_bf(xemb6), etab9=_bf(etab9), w1t=_bf(w1t), b1t=b1t,
                w2s=_bf(w2s), b2t=b2t)


def _wrap16(a):
    """Element i -> [i % 16, i // 16], replicated to 128 partitions."""
    assert len(a) % 16 == 0
    w = a.reshape(-1, 16).T
    return np.ascontiguousarray(np.tile(w, (8, 1)))


def _cell_bases(cfg):
    """slot base for each (tile, window) cell, matching the call layout:
    for each block B, for each window w, tiles of B consecutively."""
    bases = np.zeros((cfg.NTILE, cfg.NBLK), np.int64)
    off = 0
    for blk in cfg.blocks:
        for w in range(cfg.NBLK):
            for t in blk:
                bases[t, w] = off
                off += cfg.CAP
    assert off == cfg.TOTSLOT
    return bases


def _schedule_core(cfg, src_g, dst_l):
    """Assign this core's edges (global src, local dst) to gather slots.

    Returns gidx [TOTSLOT] int16 (window-local gather idx; 0 for padding)
    and selT [128, TOTGRP, 128] fp8 one-hot matrices (zero rows for pads).
    """
    # hfull is split into two AllGather halves: A = rank-major concat of
    # each core's rows [0:HALF), B = rows [HALF:NPAD).  Window w in {0,1}
    # addresses A, {2,3} addresses B.
    HALF = cfg.NPAD // 2
    q = src_g // cfg.NOWN
    local = src_g - q * cfg.NOWN
    if cfg.SPLIT_AG:
        in_a = local < HALF
        halfrow = np.where(in_a, q * HALF + local, q * HALF + local - HALF)
        w = halfrow // cfg.WIN + np.where(in_a, 0, 2)
        widx = (halfrow % cfg.WIN).astype(np.int64)
    else:
        src_row = q * cfg.NPAD + local
        w = src_row // cfg.WIN
        widx = (src_row % cfg.WIN).astype(np.int64)
    assert widx.max() < 2 ** 15
    t = dst_l // 128
    j = dst_l % 128

    cell = t * cfg.NBLK + w
    order = np.argsort(cell, kind="stable")
    cell_s = cell[order]
    counts = np.bincount(cell_s, minlength=cfg.NTILE * cfg.NBLK)
    assert counts.max() <= cfg.CAP, f"cell overflow: {counts.max()} > {cfg.CAP}"
    bases = _cell_bases(cfg).reshape(-1)
    cell_starts = np.zeros(cfg.NTILE * cfg.NBLK + 1, np.int64)
    np.cumsum(counts, out=cell_starts[1:])
    pos_in_cell = np.arange(len(cell_s)) - cell_starts[cell_s]
    slot = bases[cell_s] + pos_in_cell

    gidx = np.zeros(cfg.TOTSLOT, np.int16)
    gidx[slot] = widx[order].astype(np.int16)
    selT = np.zeros((128, cfg.TOTGRP, 128), ml_dtypes.float8_e4m3fn)
    selT[slot % 128, slot // 128, j[order]] = 1.0
    return gidx, selT


def _prepare_inputs(cfg, x, edge_index, edge_attr):
    """Per-core index/feature-side host prep."""
    x = np.asarray(x)
    src = np.asarray(edge_index[0], np.int64)
    dst = np.asarray(edge_index[1], np.int64)
    eb = np.asarray(edge_attr[:, 0], np.int64)
    ed = np.asarray(edge_attr[:, 1], np.int64)

    per_core = []
    owner = dst // cfg.NOWN
    for r in range(cfg.NCORE):
        m = owner == r
        dst_l = dst[m] - r * cfg.NOWN
        gidx, selT = _schedule_core(cfg, src[m], dst_l)

        # counts[j, n]: incoming (incl self-loop) bond-type/direction counts
        countsT = np.zeros((16, cfg.NPAD), np.float32)
        np.add.at(countsT, (eb[m], dst_l), 1.0)
        np.add.at(countsT, (6 + ed[m], dst_l), 1.0)
        loc = np.arange(cfg.NOWN)
        countsT[4, loc] += 1.0   # self-loop bond type 4
        countsT[6, loc] += 1.0   # self-loop direction 0

        xohT = np.zeros((8, cfg.NPAD), np.float32)
        xl = np.asarray(x[r * cfg.NOWN:(r + 1) * cfg.NOWN], np.int64)
        xohT[xl[:, 0], loc] = 1.0
        xohT[3 + xl[:, 1], loc] += 1.0

        per_core.append(dict(
            gidx=_wrap16(gidx), selT=selT,
            countsT=_bf(countsT), xohT=_bf(xohT),
        ))
    return per_core


def _build_program(cfg):
    nc = bacc.Bacc(None, target_bir_lowering=False, debug=True)
    f32, bf16, i16 = mybir.dt.float32, mybir.dt.bfloat16, mybir.dt.int16
    fp8 = mybir.dt.float8e4
    D, L = cfg.D, cfg.L
    GPC = cfg.GPC

    # I/O
    gidx_in = nc.dram_tensor("gidx", [128, cfg.TOTSLOT // 16], i16,
                             kind="ExternalInput")
    selT_in = nc.dram_tensor("selT", [128, cfg.TOTGRP, 128], fp8,
                             kind="ExternalInput")
    countsT_in = nc.dram_tensor("countsT", [16, cfg.NPAD], bf16,
                                kind="ExternalInput")
    xohT_in = nc.dram_tensor("xohT", [8, cfg.NPAD], bf16, kind="ExternalInput")
    xemb6_in = nc.dram_tensor("xemb6", [8, D], bf16, kind="ExternalInput")
    etab9_in = nc.dram_tensor("etab9", [L, 16, D], bf16, kind="ExternalInput")
    w1t_in = nc.dram_tensor("w1t", [L, D, 2 * D], bf16, kind="ExternalInput")
    b1t_in = nc.dram_tensor("b1t", [L, D, 2], f32, kind="ExternalInput")
    w2s_in = nc.dram_tensor("w2s", [L, 128, 2, 128], bf16, kind="ExternalInput")
    b2t_in = nc.dram_tensor("b2t", [L, D, 1], f32, kind="ExternalInput")
    out_ext = nc.dram_tensor("out", [cfg.NPAD, D], f32, kind="ExternalOutput")
    if cfg.DEBUG_TAPS:
        dbg_hown0 = nc.dram_tensor("dbg_hown0", [cfg.NPAD, D], f32,
                                   kind="ExternalOutput")
        dbg_hfA = nc.dram_tensor("dbg_hfA", [cfg.NCORE * (cfg.NPAD // 2), D],
                                 f32, kind="ExternalOutput")
        dbg_agg = nc.dram_tensor("dbg_agg", [cfg.NPAD, D], f32,
                                 kind="ExternalOutput")
        dbg_hown1 = nc.dram_tensor("dbg_hown1", [cfg.NPAD, D], f32,
                                   kind="ExternalOutput")

    # internal DRAM (shared across layers; layers are serial)
    HALF = cfg.NPAD // 2
    if cfg.SPLIT_AG:
        hownA = nc.dram_tensor("hownA", [HALF, D], bf16)
        hownB = nc.dram_tensor("hownB", [HALF, D], bf16)
        hfullA = nc.dram_tensor("hfullA", [cfg.NCORE * HALF, D], bf16,
                                addr_space="Shared")
        hfullB = nc.dram_tensor("hfullB", [cfg.NCORE * HALF, D], bf16,
                                addr_space="Shared")
        hown_d = None
    else:
        hown_d = nc.dram_tensor("hown", [cfg.NPAD, D], bf16)
        hfull = nc.dram_tensor("hfull", [cfg.HFULL, D], bf16,
                               addr_space="Shared")
        hfullA = hfullB = None

    def hown_rows(t):
        # DMA destination for the h rows of tile t (128 rows)
        r0 = t * 128
        if not cfg.SPLIT_AG:
            return hown_d[r0:r0 + 128, :]
        if r0 < HALF:
            return hownA[r0:r0 + 128, :]
        return hownB[r0 - HALF:r0 - HALF + 128, :]

    relu = mybir.ActivationFunctionType.Relu

    with tile.TileContext(nc) as tc:
        with (
            tc.tile_pool(name="const", bufs=1) as const_pool,
            tc.tile_pool(name="gather", bufs=4) as gather_pool,
            tc.tile_pool(name="sel", bufs=4) as sel_pool,
            tc.tile_pool(name="mlp", bufs=3) as mlp_pool,
            tc.tile_pool(name="aggp", bufs=2, space="PSUM") as agg_pool,
            tc.tile_pool(name="psA", bufs=1, space="PSUM") as psA_pool,
            tc.tile_pool(name="psB", bufs=2, space="PSUM") as psB_pool,
        ):
            # ---- resident constants ----
            identf = const_pool.tile([128, 128], f32, tag="identf")
            masks.make_identity(nc, identf[:, :])
            identb = const_pool.tile([128, 128], bf16, tag="identb")
            nc.vector.tensor_copy(identb[:, :], identf[:, :])
            zerob = const_pool.tile([128, cfg.TPB, 128], bf16, tag="zerob")
            nc.gpsimd.memset(zerob[:, :, :], 0.0)
            agg_sb = const_pool.tile([128, cfg.NTILE, 128], f32, tag="agg_sb")
            gidx_t = const_pool.tile([128, cfg.TOTSLOT // 16], i16, tag="gidx")
            nc.sync.dma_start(gidx_t[:, :], gidx_in[:, :])
            cntT = const_pool.tile([16, cfg.NPAD], bf16, tag="cntT")
            nc.sync.dma_start(cntT[:, :], countsT_in[:, :])
            hown_sb = const_pool.tile([128, cfg.NTILE, 128], bf16, tag="hown_sb")
            xemb6 = const_pool.tile([8, D], bf16, tag="xemb6")
            nc.sync.dma_start(xemb6[:, :], xemb6_in[:, :])
            etab9 = [const_pool.tile([16, D], bf16, tag=f"etab9_{l}",
                                     name=f"etab9_{l}") for l in range(L)]
            w1t = [const_pool.tile([D, 2 * D], bf16, tag=f"w1t_{l}",
                                   name=f"w1t_{l}") for l in range(L)]
            b1t = [const_pool.tile([D, 2], f32, tag=f"b1t_{l}",
                                   name=f"b1t_{l}") for l in range(L)]
            w2s = [const_pool.tile([128, 2, 128], bf16, tag=f"w2s_{l}",
                                   name=f"w2s_{l}") for l in range(L)]
            b2t = [const_pool.tile([D, 1], f32, tag=f"b2t_{l}",
                                   name=f"b2t_{l}") for l in range(L)]
            for l in range(L):
                nc.sync.dma_start(etab9[l][:, :], etab9_in[l])
                nc.sync.dma_start(w1t[l][:, :], w1t_in[l])
                nc.sync.dma_start(b1t[l][:, :], b1t_in[l])
                nc.sync.dma_start(w2s[l][:, :, :], w2s_in[l])
                nc.sync.dma_start(b2t[l][:, :], b2t_in[l])

            # ---- layer-0 node embedding: h0 = onehot @ xemb6 ----
            for t in range(cfg.NTILE):
                cols = slice(t * 128, (t + 1) * 128)
                xoh_t = mlp_pool.tile([8, 128], bf16, tag="xoh_t")
                nc.sync.dma_start(xoh_t[:, :], xohT_in[:, cols])
                h0p = psA_pool.tile([128, D], f32, tag="tp")
                nc.tensor.matmul(h0p[:, :], xoh_t[:, :], xemb6[:, :],
                                 start=True, stop=True)
                nc.vector.tensor_copy(hown_sb[:, t, :], h0p[:, :])
                nc.sync.dma_start(hown_rows(t), hown_sb[:, t, :])
                if cfg.DEBUG_TAPS and "hown0" in cfg.TAPS:
                    dt0 = mlp_pool.tile([128, D], f32, tag="dbg")
                    nc.vector.tensor_copy(dt0[:, :], hown_sb[:, t, :])
                    nc.sync.dma_start(dbg_hown0[cols, :], dt0[:, :])

            # ---- layers (window-major: AG halves overlap gather passes) ----
            # slot layout must match _cell_bases: block-major, window-minor
            bases = []
            off = 0
            for blk in cfg.blocks:
                row = []
                for w in range(cfg.NBLK):
                    row.append(off)
                    off += len(blk) * cfg.CAP
                bases.append(row)
            assert off == cfg.TOTSLOT

            for l in range(L):
                if cfg.SPLIT_AG:
                    nc.gpsimd.collective_compute(
                        "AllGather", mybir.AluOpType.bypass,
                        ins=[hownA[:, :]], outs=[hfullA[:, :]],
                        replica_groups=[list(range(cfg.NCORE))],
                    )
                    nc.gpsimd.collective_compute(
                        "AllGather", mybir.AluOpType.bypass,
                        ins=[hownB[:, :]], outs=[hfullB[:, :]],
                        replica_groups=[list(range(cfg.NCORE))],
                    )
                else:
                    nc.gpsimd.collective_compute(
                        "AllGather", mybir.AluOpType.bypass,
                        ins=[hown_d[:, :]], outs=[hfull[:, :]],
                        replica_groups=[list(range(cfg.NCORE))],
                    )
                if cfg.DEBUG_TAPS and l == 0 and "hfA" in cfg.TAPS:
                    for i in range(cfg.NCORE * HALF // 128):
                        rows = slice(i * 128, (i + 1) * 128)
                        db = mlp_pool.tile([128, D], bf16, tag="dbgb")
                        nc.sync.dma_start(db[:, :], hfullA[rows, :])
                        df = mlp_pool.tile([128, D], f32, tag="dbg")
                        nc.vector.tensor_copy(df[:, :], db[:, :])
                        nc.sync.dma_start(dbg_hfA[rows, :], df[:, :])
                for w in range(cfg.NBLK):
                    if cfg.SPLIT_AG:
                        src = hfullA if w < 2 else hfullB
                        woff = (w % 2) * cfg.WIN
                    else:
                        src = hfull
                        woff = w * cfg.WIN
                    for bi, blk in enumerate(cfg.blocks):
                        nt = len(blk)
                        ng_call = nt * GPC
                        nidx = nt * cfg.CAP
                        base = bases[bi][w]
                        gbuf = gather_pool.tile([128, cfg.TPB * GPC, D], bf16,
                                                tag="g")
                        ic = slice(base // 16, (base + nidx) // 16)
                        nc.gpsimd.dma_gather(
                            gbuf[:, 0:ng_call, :],
                            src[woff:woff + cfg.WIN, :],
                            gidx_t[:, ic], nidx, nidx, D,
                            single_packet=False, queue_num=0)
                        sel_t = sel_pool.tile([128, cfg.TPB * GPC, 128], fp8,
                                              tag="sel")
                        gsl = slice(base // 128, (base + nidx) // 128)
                        nc.sync.dma_start(sel_t[:, 0:ng_call, :],
                                          selT_in[:, gsl, :])
                        agg = agg_pool.tile([128, cfg.TPB, 128], f32, tag="agg")
                        if w == 0:
                            # bank-filling first mm: self-loop identity
                            nc.tensor.matmul(
                                agg[:, 0:nt, :], identb[:, :],
                                hown_sb[:, blk[0]:blk[0] + nt, :],
                                start=True, stop=False, skip_group_check=True)
                            for i, t in enumerate(blk):
                                nc.tensor.matmul(
                                    agg[:, i, :],
                                    cntT[:, t * 128:(t + 1) * 128],
                                    etab9[l][:, :],
                                    start=False, stop=False,
                                    skip_group_check=True)
                        else:
                            # bank-clearing first mm (zero rhs)
                            nc.tensor.matmul(
                                agg[:, 0:nt, :], identb[:, :],
                                zerob[:, 0:nt, :],
                                start=True, stop=False, skip_group_check=True)
                        for g in range(ng_call):
                            last = g == ng_call - 1
                            nc.tensor.matmul(
                                agg[:, g // GPC, :], sel_t[:, g, :],
                                gbuf[:, g, :],
                                start=False, stop=last, skip_group_check=True)
                        # accumulate into SBUF
                        cols = slice(blk[0], blk[0] + nt)
                        if w == 0:
                            nc.vector.tensor_copy(agg_sb[:, cols, :],
                                                  agg[:, 0:nt, :])
                        else:
                            nc.vector.tensor_add(agg_sb[:, cols, :],
                                                 agg_sb[:, cols, :],
                                                 agg[:, 0:nt, :])

                        if w == cfg.NBLK - 1:
                            if cfg.DEBUG_TAPS and l == 0 and "agg" in cfg.TAPS:
                                for t in blk:
                                    rows = slice(t * 128, (t + 1) * 128)
                                    da = mlp_pool.tile([128, D], f32,
                                                       tag="dbg")
                                    nc.vector.tensor_copy(da[:, :],
                                                          agg_sb[:, t, :])
                                    nc.sync.dma_start(dbg_agg[rows, :],
                                                      da[:, :])
                            # ---- MLP per tile of this block ----
                            for i, t in enumerate(blk):
                                tp = psA_pool.tile([128, D], f32, tag="tp")
                                nc.tensor.transpose(tp[:, :], agg_sb[:, t, :],
                                                    identf[:, :])
                                tS = mlp_pool.tile([128, D], bf16, tag="tS")
                                nc.vector.tensor_copy(tS[:, :], tp[:, :])
                                # mm1 + relu + b1
                                hm = psB_pool.tile([128, 2, 128], f32, tag="hm")
                                hmS = mlp_pool.tile([128, 2, 128], bf16,
                                                    tag="hmS")
                                for j in range(2):
                                    nc.tensor.matmul(
                                        hm[:, j, :],
                                        w1t[l][:, j * 128:(j + 1) * 128],
                                        tS[:, :], start=True, stop=True)
                                    nc.scalar.activation(
                                        hmS[:, j, :], hm[:, j, :], relu,
                                        bias=b1t[l][:, j:j + 1])
                                # mm2 accumulate + bias (+relu if not last)
                                h2p = psA_pool.tile([128, D], f32, tag="h2p")
                                for j in range(2):
                                    nc.tensor.matmul(
                                        h2p[:, :], w2s[l][:, j, :],
                                        hmS[:, j, :],
                                        start=(j == 0), stop=(j == 1))
                                if l < L - 1:
                                    h2S = mlp_pool.tile([128, D], bf16,
                                                        tag="h2S")
                                    nc.scalar.activation(
                                        h2S[:, :], h2p[:, :], relu,
                                        bias=b2t[l][:, 0:1])
                                    op = psA_pool.tile([128, D], bf16,
                                                       tag="opb")
                                    nc.tensor.transpose(op[:, :], h2S[:, :],
                                                        identb[:, :])
                                    nc.vector.tensor_copy(hown_sb[:, t, :],
                                                          op[:, :])
                                    nc.sync.dma_start(
                                        hown_rows(t), hown_sb[:, t, :])
                                    if (cfg.DEBUG_TAPS
                                            and l == 0
                                            and "hown1" in cfg.TAPS):
                                        dh = mlp_pool.tile([128, D], f32,
                                                           tag="dbg")
                                        nc.vector.tensor_copy(
                                            dh[:, :], hown_sb[:, t, :])
                                        nc.sync.dma_start(
                                            dbg_hown1[t * 128:(t + 1) * 128,
                                                      :], dh[:, :])
                                else:
                                    h2S = mlp_pool.tile([128, D], f32,
                                                        tag="h2Sf")
                                    nc.vector.tensor_scalar_add(
                                        h2S[:, :], h2p[:, :], b2t[l][:, 0:1])
                                    op = psA_pool.tile([128, D], f32,
                                                       tag="opf")
                                    nc.tensor.transpose(op[:, :], h2S[:, :],
                                                        identf[:, :])
                                    oS = mlp_pool.tile([128, D], f32,
                                                       tag="oSf")
                                    nc.vector.tensor_copy(oS[:, :], op[:, :])
                                    nc.sync.dma_start(
                                        out_ext[t * 128:(t + 1) * 128, :],
                                        oS[:, :])

    nc.finalize()
    return nc


_CACHE = {}


def _get_program(cfg):
    key = (cfg.N, cfg.CAP, cfg.TPB)
    if key not in _CACHE:
        _CACHE[key] = _build_program(cfg)
    return _CACHE[key]


def build_in_maps(cfg, inputs):
    params = _fold_params(
        cfg, inputs["x_emb"], inputs["etab"], inputs["w1"], inputs["b1"],
        inputs["w2"], inputs["b2"], inputs["gamma"], inputs["beta"],
        inputs["bn_mean"], inputs["bn_var"])
    per_core = _prepare_inputs(cfg, inputs["x"], inputs["edge_index"],
                               inputs["edge_attr"])
    in_maps = []
    for r in range(cfg.NCORE):
        m = dict(per_core[r])
        m.update({k: np.ascontiguousarray(v) for k, v in params.items()})
        in_maps.append(m)
    return in_maps


def kernel(**inputs) -> np.ndarray:
    cfg = CFG()
    nc = _get_program(cfg)
    in_maps = build_in_maps(cfg, inputs)
    res = run_bass_kernel_spmd(nc, in_maps, list(range(cfg.NCORE)))
    out = np.empty((cfg.N, cfg.D), np.float32)
    for r in range(cfg.NCORE):
        out[r * cfg.NOWN:(r + 1) * cfg.NOWN] = res.results[r]["out"][:cfg.NOWN]
    return out


# revision 23
# speedup vs baseline: 1.9487x; 1.0002x over previous
"""GIN-style 5-layer GNN message passing on 8 Trainium2 NeuronCores.

Strategy v2 (1D node-parallel, scatter-free):
  - Nodes partitioned contiguously across 8 cores (12500 each, padded to
    12544 = 98*128). Edges owned by their dst core.
  - Per layer: AllGather the per-core h shards (bf16) into a full node
    table in DRAM; dma_gather h[src] rows per edge (256B each) in
    dst-tile-sorted order, 2560 idxs per call.
  - Aggregation runs on the tensor engine, not the DMA scatter path:
    per 128-edge group (all edges of one dst tile), a one-hot fp8
    selection matrix contracts the gathered bf16 messages into a PSUM
    accumulator (out[dst, feat] += sel^T @ msg).  The self-loop term is
    the bank-filling first matmul (identity x h_own slab, start=True);
    edge embeddings fold into a counts matmul (counts^T @ etab).
  - GIN MLP (D->2D->relu->D) + BatchNorm folded into the second linear,
    bf16 weights, computed per 128-node tile on the tensor engine.
"""
import sys
import numpy as np

sys.path.insert(0, "/opt/trn_rl_repo")

import ml_dtypes
import concourse.bass as bass
import concourse.bacc as bacc
import concourse.tile as tile
import concourse.masks as masks
from concourse import mybir
from concourse.bass_utils import run_bass_kernel_spmd


class CFG:
    DEBUG_TAPS = False
    TAPS = ()
    SPLIT_AG = True
    N = 100000          # total nodes
    D = 128             # feature dim
    L = 5               # layers
    NCORE = 8
    NOWN = 12500        # nodes per core
    NPAD = 12544        # padded nodes per core (98 * 128)
    NBLK = 4            # gather source windows (int16 idx limit)
    TPB = 4             # dst tiles per block
    CAP = 640           # slot capacity per (tile, window) cell; 5 groups
    EPS = 1e-5

    @property
    def WIN(self):      # rows per gather window in h_full space
        return 2 * self.NPAD

    @property
    def NTILE(self):
        return self.NPAD // 128

    @property
    def HFULL(self):
        return self.NCORE * self.NPAD

    @property
    def GPC(self):      # groups per cell
        return self.CAP // 128

    @property
    def blocks(self):   # list of tile-index lists
        t = list(range(self.NTILE))
        return [t[i:i + self.TPB] for i in range(0, self.NTILE, self.TPB)]

    @property
    def TOTSLOT(self):
        return self.NTILE * self.NBLK * self.CAP

    @property
    def TOTGRP(self):
        return self.TOTSLOT // 128


def _f8(a):
    return np.asarray(a, np.float32).astype(ml_dtypes.float8_e4m3fn)


def _bf(a):
    return np.asarray(a, np.float32).astype(ml_dtypes.bfloat16)


def _fold_params(cfg, x_emb, etab, w1, b1, w2, b2, gamma, beta, bn_mean, bn_var):
    """Host-side parameter folding. Returns replicated device param arrays."""
    D, L = cfg.D, cfg.L
    x_emb = np.asarray(x_emb, np.float64)
    etab = np.asarray(etab, np.float64)
    w1 = np.asarray(w1, np.float64)
    b1 = np.asarray(b1, np.float64)
    w2 = np.asarray(w2, np.float64)
    b2 = np.asarray(b2, np.float64)
    gamma = np.asarray(gamma, np.float64)
    beta = np.asarray(beta, np.float64)
    bn_mean = np.asarray(bn_mean, np.float64)
    bn_var = np.asarray(bn_var, np.float64)

    xemb6 = np.zeros((8, D), np.float64)
    xemb6[0:3] = x_emb[0:3]
    xemb6[3:6] = x_emb[120:123]

    etab9 = np.zeros((L, 16, D), np.float64)
    etab9[:, 0:9, :] = etab

    w1t = np.ascontiguousarray(np.transpose(w1, (0, 2, 1)))          # [L,D,2D]
    b1t = np.ascontiguousarray(
        b1.reshape(L, 2, D).transpose(0, 2, 1)).astype(np.float32)   # [L,D,2]

    s = gamma / np.sqrt(bn_var + cfg.EPS)          # [L, D]
    t = beta - bn_mean * s
    w2f = w2 * s[:, :, None]                       # [L, D, 2D] rows scaled
    b2f = b2 * s + t                               # [L, D]
    # stationary chunks: w2s[l, p, k, m] = w2f[l, m, k*128 + p]
    w2s = np.ascontiguousarray(
        np.transpose(w2f.reshape(L, D, 2, D), (0, 3, 2, 1)))         # [L,128,2,128]
    b2t = b2f.astype(np.float32).reshape(L, D, 1)
    return dict(xemb6=_bf(xemb6), etab9=_bf(etab9), w1t=_bf(w1t), b1t=b1t,
                w2s=_bf(w2s), b2t=b2t)


def _wrap16(a):
    """Element i -> [i % 16, i // 16], replicated to 128 partitions."""
    assert len(a) % 16 == 0
    w = a.reshape(-1, 16).T
    return np.ascontiguousarray(np.tile(w, (8, 1)))


def _cell_bases(cfg):
    """slot base for each (tile, window) cell, matching the call layout:
    for each block B, for each window w, tiles of B consecutively."""
    bases = np.zeros((cfg.NTILE, cfg.NBLK), np.int64)
    off = 0
    for blk in cfg.blocks:
        for w in range(cfg.NBLK):
            for t in blk:
                bases[t, w] = off
                off += cfg.CAP
    assert off == cfg.TOTSLOT
    return bases


def _schedule_core(cfg, src_g, dst_l):
    """Assign this core's edges (global src, local dst) to gather slots.

    Returns gidx [TOTSLOT] int16 (window-local gather idx; 0 for padding)
    and selT [128, TOTGRP, 128] fp8 one-hot matrices (zero rows for pads).
    """
    # hfull is split into two AllGather halves: A = rank-major concat of
    # each core's rows [0:HALF), B = rows [HALF:NPAD).  Window w in {0,1}
    # addresses A, {2,3} addresses B.
    HALF = cfg.NPAD // 2
    q = src_g // cfg.NOWN
    local = src_g - q * cfg.NOWN
    if cfg.SPLIT_AG:
        in_a = local < HALF
        halfrow = np.where(in_a, q * HALF + local, q * HALF + local - HALF)
        w = halfrow // cfg.WIN + np.where(in_a, 0, 2)
        widx = (halfrow % cfg.WIN).astype(np.int64)
    else:
        src_row = q * cfg.NPAD + local
        w = src_row // cfg.WIN
        widx = (src_row % cfg.WIN).astype(np.int64)
    assert widx.max() < 2 ** 15
    t = dst_l // 128
    j = dst_l % 128

    cell = t * cfg.NBLK + w
    order = np.argsort(cell, kind="stable")
    cell_s = cell[order]
    counts = np.bincount(cell_s, minlength=cfg.NTILE * cfg.NBLK)
    assert counts.max() <= cfg.CAP, f"cell overflow: {counts.max()} > {cfg.CAP}"
    bases = _cell_bases(cfg).reshape(-1)
    cell_starts = np.zeros(cfg.NTILE * cfg.NBLK + 1, np.int64)
    np.cumsum(counts, out=cell_starts[1:])
    pos_in_cell = np.arange(len(cell_s)) - cell_starts[cell_s]
    slot = bases[cell_s] + pos_in_cell

    gidx = np.zeros(cfg.TOTSLOT, np.int16)
    gidx[slot] = widx[order].astype(np.int16)
    selT = np.zeros((128, cfg.TOTGRP, 128), ml_dtypes.float8_e4m3fn)
    selT[slot % 128, slot // 128, j[order]] = 1.0
    return gidx, selT


def _prepare_inputs(cfg, x, edge_index, edge_attr):
    """Per-core index/feature-side host prep."""
    x = np.asarray(x)
    src = np.asarray(edge_index[0], np.int64)
    dst = np.asarray(edge_index[1], np.int64)
    eb = np.asarray(edge_attr[:, 0], np.int64)
    ed = np.asarray(edge_attr[:, 1], np.int64)

    per_core = []
    owner = dst // cfg.NOWN
    for r in range(cfg.NCORE):
        m = owner == r
        dst_l = dst[m] - r * cfg.NOWN
        gidx, selT = _schedule_core(cfg, src[m], dst_l)

        # counts[j, n]: incoming (incl self-loop) bond-type/direction counts
        countsT = np.zeros((16, cfg.NPAD), np.float32)
        np.add.at(countsT, (eb[m], dst_l), 1.0)
        np.add.at(countsT, (6 + ed[m], dst_l), 1.0)
        loc = np.arange(cfg.NOWN)
        countsT[4, loc] += 1.0   # self-loop bond type 4
        countsT[6, loc] += 1.0   # self-loop direction 0

        xohT = np.zeros((8, cfg.NPAD), np.float32)
        xl = np.asarray(x[r * cfg.NOWN:(r + 1) * cfg.NOWN], np.int64)
        xohT[xl[:, 0], loc] = 1.0
        xohT[3 + xl[:, 1], loc] += 1.0

        per_core.append(dict(
            gidx=_wrap16(gidx), selT=selT,
            countsT=_bf(countsT), xohT=_bf(xohT),
        ))
    return per_core


def _build_program(cfg):
    nc = bacc.Bacc(None, target_bir_lowering=False, debug=True)
    f32, bf16, i16 = mybir.dt.float32, mybir.dt.bfloat16, mybir.dt.int16
    fp8 = mybir.dt.float8e4
    D, L = cfg.D, cfg.L
    GPC = cfg.GPC

    # I/O
    gidx_in = nc.dram_tensor("gidx", [128, cfg.TOTSLOT // 16], i16,
                             kind="ExternalInput")
    selT_in = nc.dram_tensor("selT", [128, cfg.TOTGRP, 128], fp8,
                             kind="ExternalInput")
    countsT_in = nc.dram_tensor("countsT", [16, cfg.NPAD], bf16,
                                kind="ExternalInput")
    xohT_in = nc.dram_tensor("xohT", [8, cfg.NPAD], bf16, kind="ExternalInput")
    xemb6_in = nc.dram_tensor("xemb6", [8, D], bf16, kind="ExternalInput")
    etab9_in = nc.dram_tensor("etab9", [L, 16, D], bf16, kind="ExternalInput")
    w1t_in = nc.dram_tensor("w1t", [L, D, 2 * D], bf16, kind="ExternalInput")
    b1t_in = nc.dram_tensor("b1t", [L, D, 2], f32, kind="ExternalInput")
    w2s_in = nc.dram_tensor("w2s", [L, 128, 2, 128], bf16, kind="ExternalInput")
    b2t_in = nc.dram_tensor("b2t", [L, D, 1], f32, kind="ExternalInput")
    out_ext = nc.dram_tensor("out", [cfg.NPAD, D], f32, kind="ExternalOutput")
    if cfg.DEBUG_TAPS:
        dbg_hown0 = nc.dram_tensor("dbg_hown0", [cfg.NPAD, D], f32,
                                   kind="ExternalOutput")
        dbg_hfA = nc.dram_tensor("dbg_hfA", [cfg.NCORE * (cfg.NPAD // 2), D],
                                 f32, kind="ExternalOutput")
        dbg_agg = nc.dram_tensor("dbg_agg", [cfg.NPAD, D], f32,
                                 kind="ExternalOutput")
        dbg_hown1 = nc.dram_tensor("dbg_hown1", [cfg.NPAD, D], f32,
                                   kind="ExternalOutput")

    # internal DRAM (shared across layers; layers are serial)
    HALF = cfg.NPAD // 2
    if cfg.SPLIT_AG:
        hownA = nc.dram_tensor("hownA", [HALF, D], bf16)
        hownB = nc.dram_tensor("hownB", [HALF, D], bf16)
        hfullA = nc.dram_tensor("hfullA", [cfg.NCORE * HALF, D], bf16,
                                addr_space="Shared")
        hfullB = nc.dram_tensor("hfullB", [cfg.NCORE * HALF, D], bf16,
                                addr_space="Shared")
        hown_d = None
    else:
        hown_d = nc.dram_tensor("hown", [cfg.NPAD, D], bf16)
        hfull = nc.dram_tensor("hfull", [cfg.HFULL, D], bf16,
                               addr_space="Shared")
        hfullA = hfullB = None

    def hown_rows(t):
        # DMA destination for the h rows of tile t (128 rows)
        r0 = t * 128
        if not cfg.SPLIT_AG:
            return hown_d[r0:r0 + 128, :]
        if r0 < HALF:
            return hownA[r0:r0 + 128, :]
        return hownB[r0 - HALF:r0 - HALF + 128, :]

    relu = mybir.ActivationFunctionType.Relu

    with tile.TileContext(nc) as tc:
        with (
            tc.tile_pool(name="const", bufs=1) as const_pool,
            tc.tile_pool(name="gather", bufs=4) as gather_pool,
            tc.tile_pool(name="sel", bufs=4) as sel_pool,
            tc.tile_pool(name="mlp", bufs=3) as mlp_pool,
            tc.tile_pool(name="aggp", bufs=2, space="PSUM") as agg_pool,
            tc.tile_pool(name="psA", bufs=1, space="PSUM") as psA_pool,
            tc.tile_pool(name="psB", bufs=2, space="PSUM") as psB_pool,
        ):
            # ---- resident constants ----
            identf = const_pool.tile([128, 128], f32, tag="identf")
            masks.make_identity(nc, identf[:, :])
            identb = const_pool.tile([128, 128], bf16, tag="identb")
            nc.vector.tensor_copy(identb[:, :], identf[:, :])
            zerob = const_pool.tile([128, cfg.TPB, 128], bf16, tag="zerob")
            nc.gpsimd.memset(zerob[:, :, :], 0.0)
            agg_sb = const_pool.tile([128, cfg.NTILE, 128], f32, tag="agg_sb")
            gidx_t = const_pool.tile([128, cfg.TOTSLOT // 16], i16, tag="gidx")
            nc.sync.dma_start(gidx_t[:, :], gidx_in[:, :])
            cntT = const_pool.tile([16, cfg.NPAD], bf16, tag="cntT")
            nc.sync.dma_start(cntT[:, :], countsT_in[:, :])
            hown_sb = const_pool.tile([128, cfg.NTILE, 128], bf16, tag="hown_sb")
            xemb6 = const_pool.tile([8, D], bf16, tag="xemb6")
            nc.sync.dma_start(xemb6[:, :], xemb6_in[:, :])
            etab9 = [const_pool.tile([16, D], bf16, tag=f"etab9_{l}",
                                     name=f"etab9_{l}") for l in range(L)]
            w1t = [const_pool.tile([D, 2 * D], bf16, tag=f"w1t_{l}",
                                   name=f"w1t_{l}") for l in range(L)]
            b1t = [const_pool.tile([D, 2], f32, tag=f"b1t_{l}",
                                   name=f"b1t_{l}") for l in range(L)]
            w2s = [const_pool.tile([128, 2, 128], bf16, tag=f"w2s_{l}",
                                   name=f"w2s_{l}") for l in range(L)]
            b2t = [const_pool.tile([D, 1], f32, tag=f"b2t_{l}",
                                   name=f"b2t_{l}") for l in range(L)]
            for l in range(L):
                nc.sync.dma_start(etab9[l][:, :], etab9_in[l])
                nc.sync.dma_start(w1t[l][:, :], w1t_in[l])
                nc.sync.dma_start(b1t[l][:, :], b1t_in[l])
                nc.sync.dma_start(w2s[l][:, :, :], w2s_in[l])
                nc.sync.dma_start(b2t[l][:, :], b2t_in[l])

            def emit_ag1():
                nc.gpsimd.collective_compute(
                    "AllGather", mybir.AluOpType.bypass,
                    ins=[hownA[:, :]], outs=[hfullA[:, :]],
                    replica_groups=[list(range(cfg.NCORE))],
                )

            def emit_ag2():
                nc.gpsimd.collective_compute(
                    "AllGather", mybir.AluOpType.bypass,
                    ins=[hownB[:, :]], outs=[hfullB[:, :]],
                    replica_groups=[list(range(cfg.NCORE))],
                )

            # ---- layer-0 node embedding: h0 = onehot @ xemb6 ----
            AG1_TILE = HALF // 128 - 1      # last tile feeding hownA (48)
            for t in range(cfg.NTILE):
                cols = slice(t * 128, (t + 1) * 128)
                xoh_t = mlp_pool.tile([8, 128], bf16, tag="xoh_t")
                nc.sync.dma_start(xoh_t[:, :], xohT_in[:, cols])
                h0p = psA_pool.tile([128, D], f32, tag="tp")
                nc.tensor.matmul(h0p[:, :], xoh_t[:, :], xemb6[:, :],
                                 start=True, stop=True)
                nc.vector.tensor_copy(hown_sb[:, t, :], h0p[:, :])
                nc.sync.dma_start(hown_rows(t), hown_sb[:, t, :])
                if cfg.SPLIT_AG and t == AG1_TILE:
                    emit_ag1()
                if cfg.DEBUG_TAPS and "hown0" in cfg.TAPS:
                    dt0 = mlp_pool.tile([128, D], f32, tag="dbg")
                    nc.vector.tensor_copy(dt0[:, :], hown_sb[:, t, :])
                    nc.sync.dma_start(dbg_hown0[cols, :], dt0[:, :])
            if cfg.SPLIT_AG:
                emit_ag2()

            # ---- layers (window-major: AG halves overlap gather passes) ----
            # slot layout must match _cell_bases: block-major, window-minor
            bases = []
            off = 0
            for blk in cfg.blocks:
                row = []
                for w in range(cfg.NBLK):
                    row.append(off)
                    off += len(blk) * cfg.CAP
                bases.append(row)
            assert off == cfg.TOTSLOT

            for l in range(L):
                # SPLIT_AG: this layer's AllGathers were already emitted
                # during the previous layer's w3 pass (or after layer 0's
                # embedding), so ncfw overlaps the remaining gather preps.
                if not cfg.SPLIT_AG:
                    nc.gpsimd.collective_compute(
                        "AllGather", mybir.AluOpType.bypass,
                        ins=[hown_d[:, :]], outs=[hfull[:, :]],
                        replica_groups=[list(range(cfg.NCORE))],
                    )
                if cfg.DEBUG_TAPS and l == 0 and "hfA" in cfg.TAPS:
                    for i in range(cfg.NCORE * HALF // 128):
                        rows = slice(i * 128, (i + 1) * 128)
                        db = mlp_pool.tile([128, D], bf16, tag="dbgb")
                        nc.sync.dma_start(db[:, :], hfullA[rows, :])
                        df = mlp_pool.tile([128, D], f32, tag="dbg")
                        nc.vector.tensor_copy(df[:, :], db[:, :])
                        nc.sync.dma_start(dbg_hfA[rows, :], df[:, :])
                for w in range(cfg.NBLK):
                    if cfg.SPLIT_AG:
                        src = hfullA if w < 2 else hfullB
                        woff = (w % 2) * cfg.WIN
                    else:
                        src = hfull
                        woff = w * cfg.WIN
                    for bi, blk in enumerate(cfg.blocks):
                        nt = len(blk)
                        ng_call = nt * GPC
                        nidx = nt * cfg.CAP
                        base = bases[bi][w]
                        gbuf = gather_pool.tile([128, cfg.TPB * GPC, D], bf16,
                                                tag="g")
                        ic = slice(base // 16, (base + nidx) // 16)
                        nc.gpsimd.dma_gather(
                            gbuf[:, 0:ng_call, :],
                            src[woff:woff + cfg.WIN, :],
                            gidx_t[:, ic], nidx, nidx, D,
                            single_packet=False, queue_num=0)
                        sel_t = sel_pool.tile([128, cfg.TPB * GPC, 128], fp8,
                                              tag="sel")
                        gsl = slice(base // 128, (base + nidx) // 128)
                        nc.sync.dma_start(sel_t[:, 0:ng_call, :],
                                          selT_in[:, gsl, :])
                        agg = agg_pool.tile([128, cfg.TPB, 128], f32, tag="agg")
                        if w == 0:
                            # bank-filling first mm: self-loop identity
                            nc.tensor.matmul(
                                agg[:, 0:nt, :], identb[:, :],
                                hown_sb[:, blk[0]:blk[0] + nt, :],
                                start=True, stop=False, skip_group_check=True)
                            for i, t in enumerate(blk):
                                nc.tensor.matmul(
                                    agg[:, i, :],
                                    cntT[:, t * 128:(t + 1) * 128],
                                    etab9[l][:, :],
                                    start=False, stop=False,
                                    skip_group_check=True)
                        else:
                            # bank-clearing first mm (zero rhs)
                            nc.tensor.matmul(
                                agg[:, 0:nt, :], identb[:, :],
                                zerob[:, 0:nt, :],
                                start=True, stop=False, skip_group_check=True)
                        for g in range(ng_call):
                            last = g == ng_call - 1
                            nc.tensor.matmul(
                                agg[:, g // GPC, :], sel_t[:, g, :],
                                gbuf[:, g, :],
                                start=False, stop=last, skip_group_check=True)
                        # accumulate into SBUF
                        cols = slice(blk[0], blk[0] + nt)
                        if w == 0:
                            nc.vector.tensor_copy(agg_sb[:, cols, :],
                                                  agg[:, 0:nt, :])
                        else:
                            nc.vector.tensor_add(agg_sb[:, cols, :],
                                                 agg_sb[:, cols, :],
                                                 agg[:, 0:nt, :])

                        if w == cfg.NBLK - 1:
                            if cfg.DEBUG_TAPS and l == 0 and "agg" in cfg.TAPS:
                                for t in blk:
                                    rows = slice(t * 128, (t + 1) * 128)
                                    da = mlp_pool.tile([128, D], f32,
                                                       tag="dbg")
                                    nc.vector.tensor_copy(da[:, :],
                                                          agg_sb[:, t, :])
                                    nc.sync.dma_start(dbg_agg[rows, :],
                                                      da[:, :])
                            # ---- MLP per tile of this block ----
                            for i, t in enumerate(blk):
                                tp = psA_pool.tile([128, D], f32, tag="tp")
                                nc.tensor.transpose(tp[:, :], agg_sb[:, t, :],
                                                    identf[:, :])
                                tS = mlp_pool.tile([128, D], bf16, tag="tS")
                                nc.vector.tensor_copy(tS[:, :], tp[:, :])
                                # mm1 + relu + b1
                                hm = psB_pool.tile([128, 2, 128], f32, tag="hm")
                                hmS = mlp_pool.tile([128, 2, 128], bf16,
                                                    tag="hmS")
                                for j in range(2):
                                    nc.tensor.matmul(
                                        hm[:, j, :],
                                        w1t[l][:, j * 128:(j + 1) * 128],
                                        tS[:, :], start=True, stop=True)
                                    nc.scalar.activation(
                                        hmS[:, j, :], hm[:, j, :], relu,
                                        bias=b1t[l][:, j:j + 1])
                                # mm2 accumulate + bias (+relu if not last)
                                h2p = psA_pool.tile([128, D], f32, tag="h2p")
                                for j in range(2):
                                    nc.tensor.matmul(
                                        h2p[:, :], w2s[l][:, j, :],
                                        hmS[:, j, :],
                                        start=(j == 0), stop=(j == 1))
                                if l < L - 1:
                                    h2S = mlp_pool.tile([128, D], bf16,
                                                        tag="h2S")
                                    nc.scalar.activation(
                                        h2S[:, :], h2p[:, :], relu,
                                        bias=b2t[l][:, 0:1])
                                    op = psA_pool.tile([128, D], bf16,
                                                       tag="opb")
                                    nc.tensor.transpose(op[:, :], h2S[:, :],
                                                        identb[:, :])
                                    nc.vector.tensor_copy(hown_sb[:, t, :],
                                                          op[:, :])
                                    nc.sync.dma_start(
                                        hown_rows(t), hown_sb[:, t, :])
                                    if (cfg.DEBUG_TAPS
                                            and l == 0
                                            and "hown1" in cfg.TAPS):
                                        dh = mlp_pool.tile([128, D], f32,
                                                           tag="dbg")
                                        nc.vector.tensor_copy(
                                            dh[:, :], hown_sb[:, t, :])
                                        nc.sync.dma_start(
                                            dbg_hown1[t * 128:(t + 1) * 128,
                                                      :], dh[:, :])
                                else:
                                    h2S = mlp_pool.tile([128, D], f32,
                                                        tag="h2Sf")
                                    nc.vector.tensor_scalar_add(
                                        h2S[:, :], h2p[:, :], b2t[l][:, 0:1])
                                    op = psA_pool.tile([128, D], f32,
                                                       tag="opf")
                                    nc.tensor.transpose(op[:, :], h2S[:, :],
                                                        identf[:, :])
                                    oS = mlp_pool.tile([128, D], f32,
                                                       tag="oSf")
                                    nc.vector.tensor_copy(oS[:, :], op[:, :])
                                    nc.sync.dma_start(
                                        out_ext[t * 128:(t + 1) * 128, :],
                                        oS[:, :])
                            # emit next layer's AllGathers as soon as their
                            # input halves are complete, so ncfw overlaps
                            # the remaining w3 gather preps
                            if cfg.SPLIT_AG and l < L - 1:
                                if AG1_TILE in blk:
                                    emit_ag1()
                                if bi == len(cfg.blocks) - 1:
                                    emit_ag2()

    nc.finalize()
    return nc


_CACHE = {}


def _get_program(cfg):
    key = (cfg.N, cfg.CAP, cfg.TPB)
    if key not in _CACHE:
        _CACHE[key] = _build_program(cfg)
    return _CACHE[key]


def build_in_maps(cfg, inputs):
    params = _fold_params(
        cfg, inputs["x_emb"], inputs["etab"], inputs["w1"], inputs["b1"],
        inputs["w2"], inputs["b2"], inputs["gamma"], inputs["beta"],
        inputs["bn_mean"], inputs["bn_var"])
    per_core = _prepare_inputs(cfg, inputs["x"], inputs["edge_index"],
                               inputs["edge_attr"])
    in_maps = []
    for r in range(cfg.NCORE):
        m = dict(per_core[r])
        m.update({k: np.ascontiguousarray(v) for k, v in params.items()})
        in_maps.append(m)
    return in_maps


def kernel(**inputs) -> np.ndarray:
    cfg = CFG()
    nc = _get_program(cfg)
    in_maps = build_in_maps(cfg, inputs)
    res = run_bass_kernel_spmd(nc, in_maps, list(range(cfg.NCORE)))
    out = np.empty((cfg.N, cfg.D), np.float32)
    for r in range(cfg.NCORE):
        out[r * cfg.NOWN:(r + 1) * cfg.NOWN] = res.results[r]["out"][:cfg.NOWN]
    return out


# revision 35
# speedup vs baseline: 2.3015x; 1.1811x over previous
"""GIN-style 5-layer GNN message passing on 8 Trainium2 NeuronCores.

Strategy v2 (1D node-parallel, scatter-free):
  - Nodes partitioned contiguously across 8 cores (12500 each, padded to
    12544 = 98*128). Edges owned by their dst core.
  - Per layer: AllGather the per-core h shards (bf16) into a full node
    table in DRAM; dma_gather h[src] rows per edge (256B each) in
    dst-tile-sorted order, 2560 idxs per call.
  - Aggregation runs on the tensor engine, not the DMA scatter path:
    per 128-edge group (all edges of one dst tile), a one-hot fp8
    selection matrix contracts the gathered bf16 messages into a PSUM
    accumulator (out[dst, feat] += sel^T @ msg).  The self-loop term is
    the bank-filling first matmul (identity x h_own slab, start=True);
    edge embeddings fold into a counts matmul (counts^T @ etab).
  - GIN MLP (D->2D->relu->D) + BatchNorm folded into the second linear,
    bf16 weights, computed per 128-node tile on the tensor engine.
"""
import sys
import numpy as np

sys.path.insert(0, "/opt/trn_rl_repo")

import ml_dtypes
import concourse.bass as bass
import concourse.bacc as bacc
import concourse.tile as tile
import concourse.masks as masks
from concourse import mybir
from concourse.bass_utils import run_bass_kernel_spmd


class CFG:
    DEBUG_TAPS = False
    TAPS = ()
    SPLIT_AG = True
    N = 100000          # total nodes
    D = 128             # feature dim
    L = 5               # layers
    NCORE = 8
    NOWN = 12500        # nodes per core
    NPAD = 12544        # padded nodes per core (98 * 128)
    NBLK = 4            # gather source windows (int16 idx limit)
    TPB = 4             # dst tiles per block
    CAP = 512           # slot capacity per (tile, window) cell; 4 groups
    # overflow sub-call capacities per window (scatter-add path);
    # each sub-call holds edges with distinct dst nodes
    OCAPS = (1024, 512, 256, 128, 128)
    EPS = 1e-5

    @property
    def OWIN(self):     # overflow slots per window
        return sum(self.OCAPS)

    @property
    def WIN(self):      # rows per gather window in h_full space
        return 2 * self.NPAD

    @property
    def NTILE(self):
        return self.NPAD // 128

    @property
    def HFULL(self):
        return self.NCORE * self.NPAD

    @property
    def GPC(self):      # groups per cell
        return self.CAP // 128

    @property
    def blocks(self):   # list of tile-index lists
        t = list(range(self.NTILE))
        return [t[i:i + self.TPB] for i in range(0, self.NTILE, self.TPB)]

    @property
    def TOTSLOT(self):
        return self.NTILE * self.NBLK * self.CAP

    @property
    def TOTGRP(self):
        return self.TOTSLOT // 128


def _f8(a):
    return np.asarray(a, np.float32).astype(ml_dtypes.float8_e4m3fn)


def _bf(a):
    return np.asarray(a, np.float32).astype(ml_dtypes.bfloat16)


def _fold_params(cfg, x_emb, etab, w1, b1, w2, b2, gamma, beta, bn_mean, bn_var):
    """Host-side parameter folding. Returns replicated device param arrays."""
    D, L = cfg.D, cfg.L
    x_emb = np.asarray(x_emb, np.float64)
    etab = np.asarray(etab, np.float64)
    w1 = np.asarray(w1, np.float64)
    b1 = np.asarray(b1, np.float64)
    w2 = np.asarray(w2, np.float64)
    b2 = np.asarray(b2, np.float64)
    gamma = np.asarray(gamma, np.float64)
    beta = np.asarray(beta, np.float64)
    bn_mean = np.asarray(bn_mean, np.float64)
    bn_var = np.asarray(bn_var, np.float64)

    xemb6 = np.zeros((8, D), np.float64)
    xemb6[0:3] = x_emb[0:3]
    xemb6[3:6] = x_emb[120:123]

    etab9 = np.zeros((L, 16, D), np.float64)
    etab9[:, 0:9, :] = etab

    w1t = np.ascontiguousarray(np.transpose(w1, (0, 2, 1)))          # [L,D,2D]
    b1t = np.ascontiguousarray(
        b1.reshape(L, 2, D).transpose(0, 2, 1)).astype(np.float32)   # [L,D,2]

    s = gamma / np.sqrt(bn_var + cfg.EPS)          # [L, D]
    t = beta - bn_mean * s
    w2f = w2 * s[:, :, None]                       # [L, D, 2D] rows scaled
    b2f = b2 * s + t                               # [L, D]
    # stationary chunks: w2s[l, p, k, m] = w2f[l, m, k*128 + p]
    w2s = np.ascontiguousarray(
        np.transpose(w2f.reshape(L, D, 2, D), (0, 3, 2, 1)))         # [L,128,2,128]
    b2t = b2f.astype(np.float32).reshape(L, D, 1)
    return dict(xemb6=_bf(xemb6), etab9=_bf(etab9), w1t=_bf(w1t), b1t=b1t,
                w2s=_bf(w2s), b2t=b2t)


def _wrap16(a):
    """Element i -> [i % 16, i // 16], replicated to 128 partitions."""
    assert len(a) % 16 == 0
    w = a.reshape(-1, 16).T
    return np.ascontiguousarray(np.tile(w, (8, 1)))


def _cell_bases(cfg):
    """slot base for each (tile, window) cell, matching the call layout:
    for each block B, for each window w, tiles of B consecutively."""
    bases = np.zeros((cfg.NTILE, cfg.NBLK), np.int64)
    off = 0
    for blk in cfg.blocks:
        for w in range(cfg.NBLK):
            for t in blk:
                bases[t, w] = off
                off += cfg.CAP
    assert off == cfg.TOTSLOT
    return bases


def _schedule_core(cfg, src_g, dst_l):
    """Assign this core's edges (global src, local dst) to gather slots.

    Returns gidx [TOTSLOT] int16 (window-local gather idx; 0 for padding)
    and selT [128, TOTGRP, 128] fp8 one-hot matrices (zero rows for pads).
    """
    # hfull is split into two AllGather halves: A = rank-major concat of
    # each core's rows [0:HALF), B = rows [HALF:NPAD).  Window w in {0,1}
    # addresses A, {2,3} addresses B.
    HALF = cfg.NPAD // 2
    q = src_g // cfg.NOWN
    local = src_g - q * cfg.NOWN
    if cfg.SPLIT_AG:
        in_a = local < HALF
        halfrow = np.where(in_a, q * HALF + local, q * HALF + local - HALF)
        w = halfrow // cfg.WIN + np.where(in_a, 0, 2)
        widx = (halfrow % cfg.WIN).astype(np.int64)
    else:
        src_row = q * cfg.NPAD + local
        w = src_row // cfg.WIN
        widx = (src_row % cfg.WIN).astype(np.int64)
    assert widx.max() < 2 ** 15
    t = dst_l // 128
    j = dst_l % 128

    cell = t * cfg.NBLK + w
    order = np.argsort(cell, kind="stable")
    cell_s = cell[order]
    counts = np.bincount(cell_s, minlength=cfg.NTILE * cfg.NBLK)
    bases = _cell_bases(cfg).reshape(-1)
    cell_starts = np.zeros(cfg.NTILE * cfg.NBLK + 1, np.int64)
    np.cumsum(counts, out=cell_starts[1:])
    pos_in_cell = np.arange(len(cell_s)) - cell_starts[cell_s]
    in_main = pos_in_cell < cfg.CAP
    slot = bases[cell_s] + pos_in_cell

    gidx = np.zeros(cfg.TOTSLOT, np.int16)
    gidx[slot[in_main]] = widx[order][in_main].astype(np.int16)
    selT = np.zeros((128, cfg.TOTGRP, 128), ml_dtypes.float8_e4m3fn)
    selT[slot[in_main] % 128, slot[in_main] // 128,
         j[order][in_main]] = 1.0

    # ---- overflow edges -> per-window scatter-add sub-calls ----
    # slot layout: [window][sub-call][cap]; within each sub-call dst
    # nodes are distinct.  Unused slots scatter row 0 of the window into
    # trash rows >= NPAD (negative "ignored" indices fault on this HW).
    nslot = cfg.NBLK * cfg.OWIN
    gidxo = np.zeros(nslot, np.int16)
    sidxo = (cfg.NPAD + np.arange(nslot) % 1024).astype(np.int16)
    ov = ~in_main
    ov_w = (cell_s % cfg.NBLK)[ov]
    ov_widx = widx[order][ov]
    ov_dst = dst_l[order][ov]
    for wv in range(cfg.NBLK):
        m = ov_w == wv
        dsts = ov_dst[m]
        wis = ov_widx[m]
        fill = [0] * len(cfg.OCAPS)
        used = [set() for _ in cfg.OCAPS]
        woff = wv * cfg.OWIN
        for dd, wi in zip(dsts.tolist(), wis.tolist()):
            for k, cap in enumerate(cfg.OCAPS):
                if fill[k] < cap and dd not in used[k]:
                    off = woff + sum(cfg.OCAPS[:k]) + fill[k]
                    gidxo[off] = wi
                    sidxo[off] = dd
                    used[k].add(dd)
                    fill[k] += 1
                    break
            else:
                raise AssertionError("overflow sub-calls exhausted")
    # gather pads must be valid indices (interior positions); the
    # matching scatter slots stay -1 so their values are never used
    gpad = gidxo < 0
    gidxo[gpad] = 0
    return gidx, selT, gidxo, sidxo


def _prepare_inputs(cfg, x, edge_index, edge_attr):
    """Per-core index/feature-side host prep."""
    x = np.asarray(x)
    src = np.asarray(edge_index[0], np.int64)
    dst = np.asarray(edge_index[1], np.int64)
    eb = np.asarray(edge_attr[:, 0], np.int64)
    ed = np.asarray(edge_attr[:, 1], np.int64)

    per_core = []
    owner = dst // cfg.NOWN
    for r in range(cfg.NCORE):
        m = owner == r
        dst_l = dst[m] - r * cfg.NOWN
        gidx, selT, gidxo, sidxo = _schedule_core(cfg, src[m], dst_l)

        # counts[j, n]: incoming (incl self-loop) bond-type/direction counts
        countsT = np.zeros((16, cfg.NPAD), np.float32)
        np.add.at(countsT, (eb[m], dst_l), 1.0)
        np.add.at(countsT, (6 + ed[m], dst_l), 1.0)
        loc = np.arange(cfg.NOWN)
        countsT[4, loc] += 1.0   # self-loop bond type 4
        countsT[6, loc] += 1.0   # self-loop direction 0

        xohT = np.zeros((8, cfg.NPAD), np.float32)
        xl = np.asarray(x[r * cfg.NOWN:(r + 1) * cfg.NOWN], np.int64)
        xohT[xl[:, 0], loc] = 1.0
        xohT[3 + xl[:, 1], loc] += 1.0

        per_core.append(dict(
            gidx=_wrap16(gidx), selT=selT,
            gidxo=_wrap16(gidxo), sidxo=_wrap16(sidxo),
            countsT=_bf(countsT), xohT=_bf(xohT),
        ))
    return per_core


def _build_program(cfg):
    nc = bacc.Bacc(None, target_bir_lowering=False, debug=True)
    f32, bf16, i16 = mybir.dt.float32, mybir.dt.bfloat16, mybir.dt.int16
    fp8 = mybir.dt.float8e4
    D, L = cfg.D, cfg.L
    GPC = cfg.GPC

    # I/O
    gidx_in = nc.dram_tensor("gidx", [128, cfg.TOTSLOT // 16], i16,
                             kind="ExternalInput")
    gidxo_in = nc.dram_tensor("gidxo", [128, cfg.NBLK * cfg.OWIN // 16], i16,
                              kind="ExternalInput")
    sidxo_in = nc.dram_tensor("sidxo", [128, cfg.NBLK * cfg.OWIN // 16], i16,
                              kind="ExternalInput")
    selT_in = nc.dram_tensor("selT", [128, cfg.TOTGRP, 128], fp8,
                             kind="ExternalInput")
    countsT_in = nc.dram_tensor("countsT", [16, cfg.NPAD], bf16,
                                kind="ExternalInput")
    xohT_in = nc.dram_tensor("xohT", [8, cfg.NPAD], bf16, kind="ExternalInput")
    xemb6_in = nc.dram_tensor("xemb6", [8, D], bf16, kind="ExternalInput")
    etab9_in = nc.dram_tensor("etab9", [L, 16, D], bf16, kind="ExternalInput")
    w1t_in = nc.dram_tensor("w1t", [L, D, 2 * D], bf16, kind="ExternalInput")
    b1t_in = nc.dram_tensor("b1t", [L, D, 2], f32, kind="ExternalInput")
    w2s_in = nc.dram_tensor("w2s", [L, 128, 2, 128], bf16, kind="ExternalInput")
    b2t_in = nc.dram_tensor("b2t", [L, D, 1], f32, kind="ExternalInput")
    out_ext = nc.dram_tensor("out", [cfg.NPAD, D], f32, kind="ExternalOutput")
    if cfg.DEBUG_TAPS:
        dbg_hown0 = nc.dram_tensor("dbg_hown0", [cfg.NPAD, D], f32,
                                   kind="ExternalOutput")
        dbg_hfA = nc.dram_tensor("dbg_hfA", [cfg.NCORE * (cfg.NPAD // 2), D],
                                 f32, kind="ExternalOutput")
        dbg_agg = nc.dram_tensor("dbg_agg", [cfg.NPAD, D], f32,
                                 kind="ExternalOutput")
        dbg_hown1 = nc.dram_tensor("dbg_hown1", [cfg.NPAD, D], f32,
                                   kind="ExternalOutput")

    # internal DRAM (shared across layers; layers are serial)
    HALF = cfg.NPAD // 2
    if cfg.SPLIT_AG:
        hownA = nc.dram_tensor("hownA", [HALF, D], bf16)
        hownB = nc.dram_tensor("hownB", [HALF, D], bf16)
        hfullA = nc.dram_tensor("hfullA", [cfg.NCORE * HALF, D], bf16,
                                addr_space="Shared")
        hfullB = nc.dram_tensor("hfullB", [cfg.NCORE * HALF, D], bf16,
                                addr_space="Shared")
        hown_d = None
    else:
        hown_d = nc.dram_tensor("hown", [cfg.NPAD, D], bf16)
        hfull = nc.dram_tensor("hfull", [cfg.HFULL, D], bf16,
                               addr_space="Shared")
        hfullA = hfullB = None

    ovf_d = nc.dram_tensor("ovf", [cfg.NPAD + 1024, D], bf16)

    def hown_rows(t):
        # DMA destination for the h rows of tile t (128 rows)
        r0 = t * 128
        if not cfg.SPLIT_AG:
            return hown_d[r0:r0 + 128, :]
        if r0 < HALF:
            return hownA[r0:r0 + 128, :]
        return hownB[r0 - HALF:r0 - HALF + 128, :]

    relu = mybir.ActivationFunctionType.Relu

    with tile.TileContext(nc) as tc:
        with (
            tc.tile_pool(name="const", bufs=1) as const_pool,
            tc.tile_pool(name="gather", bufs=4) as gather_pool,
            tc.tile_pool(name="sel", bufs=4) as sel_pool,
            tc.tile_pool(name="mlp", bufs=3) as mlp_pool,
            tc.tile_pool(name="aggp", bufs=2, space="PSUM") as agg_pool,
            tc.tile_pool(name="psA", bufs=1, space="PSUM") as psA_pool,
            tc.tile_pool(name="psB", bufs=2, space="PSUM") as psB_pool,
        ):
            # ---- resident constants ----
            identf = const_pool.tile([128, 128], f32, tag="identf")
            masks.make_identity(nc, identf[:, :])
            identb = const_pool.tile([128, 128], bf16, tag="identb")
            nc.vector.tensor_copy(identb[:, :], identf[:, :])
            zerob = const_pool.tile([128, cfg.TPB, 128], bf16, tag="zerob")
            nc.gpsimd.memset(zerob[:, :, :], 0.0)
            agg_sb = const_pool.tile([128, cfg.NTILE, 128], f32, tag="agg_sb")
            gidx_t = const_pool.tile([128, cfg.TOTSLOT // 16], i16, tag="gidx")
            nc.sync.dma_start(gidx_t[:, :], gidx_in[:, :])
            gidxo_t = const_pool.tile([128, cfg.NBLK * cfg.OWIN // 16], i16,
                                      tag="gidxo")
            nc.sync.dma_start(gidxo_t[:, :], gidxo_in[:, :])
            sidxo_t = const_pool.tile([128, cfg.NBLK * cfg.OWIN // 16], i16,
                                      tag="sidxo")
            nc.sync.dma_start(sidxo_t[:, :], sidxo_in[:, :])
            cntT = const_pool.tile([16, cfg.NPAD], bf16, tag="cntT")
            nc.sync.dma_start(cntT[:, :], countsT_in[:, :])
            hown_sb = const_pool.tile([128, cfg.NTILE, 128], bf16, tag="hown_sb")
            xemb6 = const_pool.tile([8, D], bf16, tag="xemb6")
            nc.sync.dma_start(xemb6[:, :], xemb6_in[:, :])
            etab9 = [const_pool.tile([16, D], bf16, tag=f"etab9_{l}",
                                     name=f"etab9_{l}") for l in range(L)]
            w1t = [const_pool.tile([D, 2 * D], bf16, tag=f"w1t_{l}",
                                   name=f"w1t_{l}") for l in range(L)]
            b1t = [const_pool.tile([D, 2], f32, tag=f"b1t_{l}",
                                   name=f"b1t_{l}") for l in range(L)]
            w2s = [const_pool.tile([128, 2, 128], bf16, tag=f"w2s_{l}",
                                   name=f"w2s_{l}") for l in range(L)]
            b2t = [const_pool.tile([D, 1], f32, tag=f"b2t_{l}",
                                   name=f"b2t_{l}") for l in range(L)]
            for l in range(L):
                nc.sync.dma_start(etab9[l][:, :], etab9_in[l])
                nc.sync.dma_start(w1t[l][:, :], w1t_in[l])
                nc.sync.dma_start(b1t[l][:, :], b1t_in[l])
                nc.sync.dma_start(w2s[l][:, :, :], w2s_in[l])
                nc.sync.dma_start(b2t[l][:, :], b2t_in[l])

            def emit_ag1():
                nc.gpsimd.collective_compute(
                    "AllGather", mybir.AluOpType.bypass,
                    ins=[hownA[:, :]], outs=[hfullA[:, :]],
                    replica_groups=[list(range(cfg.NCORE))],
                )

            def emit_ag2():
                nc.gpsimd.collective_compute(
                    "AllGather", mybir.AluOpType.bypass,
                    ins=[hownB[:, :]], outs=[hfullB[:, :]],
                    replica_groups=[list(range(cfg.NCORE))],
                )

            # ---- layer-0 node embedding: h0 = onehot @ xemb6 ----
            AG1_TILE = HALF // 128 - 1      # last tile feeding hownA (48)
            for t in range(cfg.NTILE):
                cols = slice(t * 128, (t + 1) * 128)
                xoh_t = mlp_pool.tile([8, 128], bf16, tag="xoh_t")
                nc.sync.dma_start(xoh_t[:, :], xohT_in[:, cols])
                h0p = psA_pool.tile([128, D], f32, tag="tp")
                nc.tensor.matmul(h0p[:, :], xoh_t[:, :], xemb6[:, :],
                                 start=True, stop=True)
                nc.vector.tensor_copy(hown_sb[:, t, :], h0p[:, :])
                nc.sync.dma_start(hown_rows(t), hown_sb[:, t, :])
                if cfg.SPLIT_AG and t == AG1_TILE:
                    emit_ag1()
                if cfg.DEBUG_TAPS and "hown0" in cfg.TAPS:
                    dt0 = mlp_pool.tile([128, D], f32, tag="dbg")
                    nc.vector.tensor_copy(dt0[:, :], hown_sb[:, t, :])
                    nc.sync.dma_start(dbg_hown0[cols, :], dt0[:, :])
            if cfg.SPLIT_AG:
                emit_ag2()

            # ---- layers (window-major: AG halves overlap gather passes) ----
            # slot layout must match _cell_bases: block-major, window-minor
            bases = []
            off = 0
            for blk in cfg.blocks:
                row = []
                for w in range(cfg.NBLK):
                    row.append(off)
                    off += len(blk) * cfg.CAP
                bases.append(row)
            assert off == cfg.TOTSLOT

            for l in range(L):
                # SPLIT_AG: this layer's AllGathers were already emitted
                # during the previous layer's w3 pass (or after layer 0's
                # embedding), so ncfw overlaps the remaining gather preps.
                if not cfg.SPLIT_AG:
                    nc.gpsimd.collective_compute(
                        "AllGather", mybir.AluOpType.bypass,
                        ins=[hown_d[:, :]], outs=[hfull[:, :]],
                        replica_groups=[list(range(cfg.NCORE))],
                    )
                # zero the overflow accumulator
                r0 = 0
                while r0 < cfg.NPAD:
                    rows = min(512, cfg.NPAD - r0)
                    view = ovf_d[r0:r0 + rows, :].rearrange(
                        "(i o) d -> i (o d)", i=128)
                    nc.sync.dma_start(view, zerob[:, :, :].rearrange(
                        "p a b -> p (a b)")[:, : rows * D // 128])
                    r0 += rows
                if cfg.DEBUG_TAPS and l == 0 and "hfA" in cfg.TAPS:
                    for i in range(cfg.NCORE * HALF // 128):
                        rows = slice(i * 128, (i + 1) * 128)
                        db = mlp_pool.tile([128, D], bf16, tag="dbgb")
                        nc.sync.dma_start(db[:, :], hfullA[rows, :])
                        df = mlp_pool.tile([128, D], f32, tag="dbg")
                        nc.vector.tensor_copy(df[:, :], db[:, :])
                        nc.sync.dma_start(dbg_hfA[rows, :], df[:, :])
                for w in range(cfg.NBLK):
                    if cfg.SPLIT_AG:
                        src = hfullA if w < 2 else hfullB
                        woff = (w % 2) * cfg.WIN
                    else:
                        src = hfull
                        woff = w * cfg.WIN
                    # overflow edges of this window: gather then
                    # scatter-add into the ovf accumulator (sub-calls
                    # have distinct dst rows; -1 tails are skipped)
                    ogb = gather_pool.tile([128, cfg.OWIN // 128, D], bf16,
                                           tag="og")
                    oc = slice(w * cfg.OWIN // 16, (w + 1) * cfg.OWIN // 16)
                    nc.gpsimd.dma_gather(
                        ogb[:, :, :], src[woff:woff + cfg.WIN, :],
                        gidxo_t[:, oc], cfg.OWIN, cfg.OWIN, D,
                        single_packet=False, queue_num=0)
                    ooff = 0
                    for cap in cfg.OCAPS:
                        isl = slice((w * cfg.OWIN + ooff) // 16,
                                    (w * cfg.OWIN + ooff + cap) // 16)
                        nc.gpsimd.dma_scatter_add(
                            ovf_d[:, :],
                            ogb[:, ooff // 128:(ooff + cap) // 128, :],
                            sidxo_t[:, isl], cap, cap, D, queue_num=0)
                        ooff += cap
                    for bi, blk in enumerate(cfg.blocks):
                        nt = len(blk)
                        ng_call = nt * GPC
                        nidx = nt * cfg.CAP
                        base = bases[bi][w]
                        gbuf = gather_pool.tile([128, cfg.TPB * GPC, D], bf16,
                                                tag="g")
                        ic = slice(base // 16, (base + nidx) // 16)
                        nc.gpsimd.dma_gather(
                            gbuf[:, 0:ng_call, :],
                            src[woff:woff + cfg.WIN, :],
                            gidx_t[:, ic], nidx, nidx, D,
                            single_packet=False, queue_num=0)
                        sel_t = sel_pool.tile([128, cfg.TPB * GPC, 128], fp8,
                                              tag="sel")
                        gsl = slice(base // 128, (base + nidx) // 128)
                        nc.sync.dma_start(sel_t[:, 0:ng_call, :],
                                          selT_in[:, gsl, :])
                        agg = agg_pool.tile([128, cfg.TPB, 128], f32, tag="agg")
                        if w == 0:
                            # bank-filling first mm: self-loop identity
                            nc.tensor.matmul(
                                agg[:, 0:nt, :], identb[:, :],
                                hown_sb[:, blk[0]:blk[0] + nt, :],
                                start=True, stop=False, skip_group_check=True)
                            for i, t in enumerate(blk):
                                nc.tensor.matmul(
                                    agg[:, i, :],
                                    cntT[:, t * 128:(t + 1) * 128],
                                    etab9[l][:, :],
                                    start=False, stop=False,
                                    skip_group_check=True)
                        else:
                            # bank-clearing first mm (zero rhs)
                            nc.tensor.matmul(
                                agg[:, 0:nt, :], identb[:, :],
                                zerob[:, 0:nt, :],
                                start=True, stop=False, skip_group_check=True)
                        for g in range(ng_call):
                            last = (w != cfg.NBLK - 1) and (g == ng_call - 1)
                            nc.tensor.matmul(
                                agg[:, g // GPC, :], sel_t[:, g, :],
                                gbuf[:, g, :],
                                start=False, stop=last, skip_group_check=True)
                        if w == cfg.NBLK - 1:
                            # merge the overflow accumulator (one identity
                            # matmul per tile, still into the same bank)
                            for i, t in enumerate(blk):
                                ovt = mlp_pool.tile([128, D], bf16, tag="ovt")
                                nc.sync.dma_start(
                                    ovt[:, :],
                                    ovf_d[t * 128:(t + 1) * 128, :])
                                nc.tensor.matmul(
                                    agg[:, i, :], identb[:, :], ovt[:, :],
                                    start=False, stop=(i == nt - 1),
                                    skip_group_check=True)
                        # accumulate into SBUF
                        cols = slice(blk[0], blk[0] + nt)
                        if w == 0:
                            nc.vector.tensor_copy(agg_sb[:, cols, :],
                                                  agg[:, 0:nt, :])
                        else:
                            nc.vector.tensor_add(agg_sb[:, cols, :],
                                                 agg_sb[:, cols, :],
                                                 agg[:, 0:nt, :])

                        if w == cfg.NBLK - 1:
                            if cfg.DEBUG_TAPS and l == 0 and "agg" in cfg.TAPS:
                                for t in blk:
                                    rows = slice(t * 128, (t + 1) * 128)
                                    da = mlp_pool.tile([128, D], f32,
                                                       tag="dbg")
                                    nc.vector.tensor_copy(da[:, :],
                                                          agg_sb[:, t, :])
                                    nc.sync.dma_start(dbg_agg[rows, :],
                                                      da[:, :])
                            # ---- MLP per tile of this block ----
                            for i, t in enumerate(blk):
                                tp = psA_pool.tile([128, D], f32, tag="tp")
                                nc.tensor.transpose(tp[:, :], agg_sb[:, t, :],
                                                    identf[:, :])
                                tS = mlp_pool.tile([128, D], bf16, tag="tS")
                                nc.vector.tensor_copy(tS[:, :], tp[:, :])
                                # mm1 + relu + b1
                                hm = psB_pool.tile([128, 2, 128], f32, tag="hm")
                                hmS = mlp_pool.tile([128, 2, 128], bf16,
                                                    tag="hmS")
                                for j in range(2):
                                    nc.tensor.matmul(
                                        hm[:, j, :],
                                        w1t[l][:, j * 128:(j + 1) * 128],
                                        tS[:, :], start=True, stop=True)
                                    nc.scalar.activation(
                                        hmS[:, j, :], hm[:, j, :], relu,
                                        bias=b1t[l][:, j:j + 1])
                                # mm2 accumulate + bias (+relu if not last)
                                h2p = psA_pool.tile([128, D], f32, tag="h2p")
                                for j in range(2):
                                    nc.tensor.matmul(
                                        h2p[:, :], w2s[l][:, j, :],
                                        hmS[:, j, :],
                                        start=(j == 0), stop=(j == 1))
                                if l < L - 1:
                                    h2S = mlp_pool.tile([128, D], bf16,
                                                        tag="h2S")
                                    nc.scalar.activation(
                                        h2S[:, :], h2p[:, :], relu,
                                        bias=b2t[l][:, 0:1])
                                    op = psA_pool.tile([128, D], bf16,
                                                       tag="opb")
                                    nc.tensor.transpose(op[:, :], h2S[:, :],
                                                        identb[:, :])
                                    nc.vector.tensor_copy(hown_sb[:, t, :],
                                                          op[:, :])
                                    nc.sync.dma_start(
                                        hown_rows(t), hown_sb[:, t, :])
                                    if (cfg.DEBUG_TAPS
                                            and l == 0
                                            and "hown1" in cfg.TAPS):
                                        dh = mlp_pool.tile([128, D], f32,
                                                           tag="dbg")
                                        nc.vector.tensor_copy(
                                            dh[:, :], hown_sb[:, t, :])
                                        nc.sync.dma_start(
                                            dbg_hown1[t * 128:(t + 1) * 128,
                                                      :], dh[:, :])
                                else:
                                    h2S = mlp_pool.tile([128, D], f32,
                                                        tag="h2Sf")
                                    nc.vector.tensor_scalar_add(
                                        h2S[:, :], h2p[:, :], b2t[l][:, 0:1])
                                    op = psA_pool.tile([128, D], f32,
                                                       tag="opf")
                                    nc.tensor.transpose(op[:, :], h2S[:, :],
                                                        identf[:, :])
                                    oS = mlp_pool.tile([128, D], f32,
                                                       tag="oSf")
                                    nc.vector.tensor_copy(oS[:, :], op[:, :])
                                    nc.sync.dma_start(
                                        out_ext[t * 128:(t + 1) * 128, :],
                                        oS[:, :])
                            # emit next layer's AllGathers as soon as their
                            # input halves are complete, so ncfw overlaps
                            # the remaining w3 gather preps
                            if cfg.SPLIT_AG and l < L - 1:
                                if AG1_TILE in blk:
                                    emit_ag1()
                                if bi == len(cfg.blocks) - 1:
                                    emit_ag2()

    nc.finalize()
    return nc


_CACHE = {}


def _get_program(cfg):
    key = (cfg.N, cfg.CAP, cfg.TPB)
    if key not in _CACHE:
        _CACHE[key] = _build_program(cfg)
    return _CACHE[key]


def build_in_maps(cfg, inputs):
    params = _fold_params(
        cfg, inputs["x_emb"], inputs["etab"], inputs["w1"], inputs["b1"],
        inputs["w2"], inputs["b2"], inputs["gamma"], inputs["beta"],
        inputs["bn_mean"], inputs["bn_var"])
    per_core = _prepare_inputs(cfg, inputs["x"], inputs["edge_index"],
                               inputs["edge_attr"])
    in_maps = []
    for r in range(cfg.NCORE):
        m = dict(per_core[r])
        m.update({k: np.ascontiguousarray(v) for k, v in params.items()})
        in_maps.append(m)
    return in_maps


def kernel(**inputs) -> np.ndarray:
    cfg = CFG()
    nc = _get_program(cfg)
    in_maps = build_in_maps(cfg, inputs)
    res = run_bass_kernel_spmd(nc, in_maps, list(range(cfg.NCORE)))
    out = np.empty((cfg.N, cfg.D), np.float32)
    for r in range(cfg.NCORE):
        out[r * cfg.NOWN:(r + 1) * cfg.NOWN] = res.results[r]["out"][:cfg.NOWN]
    return out
